# revision 1
# baseline (speedup 1.0000x reference)
"""DCNv3_C Trainium2 Bass kernel.

8-core data parallelism over the batch (one image per NeuronCore).
Per core: 1x1 conv -> value proj -> depthwise 3x3 (block-diag matmuls)
-> LN+gelu -> offset/mask proj -> softmax -> dense 5x5 "hat" sampling
weights -> 25-bin weighted window sum (DVE scalar_tensor_tensor)
-> output proj.

DCNv3 bilinear sampling is rewritten exactly (for |offset|<=1) as a 5x5
locally-connected window:
  acc[s,g,c] = sum_{dy,dx in [-2,2]} DW[s,g,dy,dx] * VP[s+(dy,dx), g, c]
  DW[s,g,dy,dx] = sum_p mask_p * hat(gy_p+offy_p-dy) * hat(gx_p+offx_p-dx)
with hat(t)=max(0,1-|t|) and VP the value map zero-padded by 2.
"""

import numpy as np

N, C_IN, C, H, W = 8, 192, 256, 64, 64
G, K, PAD = 4, 3, 1
GC = C // G          # 64
P = K * K            # 9
S = H * W            # 4096
NCORES = 8

_CACHE = {}
TRACE = False
_LAST_EXEC_NS = None


def _host_consts():
    # p = a*3+b with grid_x = a-1 (slowest), grid_y = b-1
    gx = np.repeat(np.arange(3) - 1, 3)
    gy = np.tile(np.arange(3) - 1, 3)
    # p-sum selection matrices, one per (xb, yb): [36, 100]
    # row (g, p) -> col g*25 + d, d = (dy+2)*5 + (dx+2)
    Smats = np.zeros((3, 3, 36, 100), np.float32)
    for xb in range(3):
        for yb in range(3):
            for g in range(G):
                for p_ in range(P):
                    dy = gy[p_] + yb - 1
                    dx = gx[p_] + xb - 1
                    d = (dy + 2) * 5 + (dx + 2)
                    Smats[xb, yb, g * 9 + p_, g * 25 + d] = 1.0
    E9 = np.zeros((36, 4), np.float32)     # per-group sums
    E9T = np.zeros((4, 36), np.float32)    # per-group broadcast
    for g in range(G):
        E9[g * 9:(g + 1) * 9, g] = 1.0
        E9T[g, g * 9:(g + 1) * 9] = 1.0
    return Smats, E9, E9T


def _prep_weights(inp):
    import ml_dtypes as _mldw
    w = {}
    w['wc'] = np.ascontiguousarray(inp['conv_w'].T).astype(np.float32)   # [192,256]
    w['bc'] = inp['conv_b'].reshape(C, 1).astype(np.float32)
    w['win'] = np.ascontiguousarray(inp['in_w'].T).astype(_mldw.bfloat16)  # [c,o]
    w['inb'] = np.asarray(inp['in_b'], np.float32)
    # depthwise diag weights, partition-major: [128, 9, 2, 128]
    import ml_dtypes as _mld0
    dwd = np.zeros((128, 9, 2, 128), np.float32)
    dw = inp['dw_w'].reshape(C, 9)
    for tap in range(9):
        for mt in range(2):
            for i in range(128):
                dwd[i, tap, mt, i] = dw[mt * 128 + i, tap]
    w['dwd'] = dwd.astype(_mld0.bfloat16)
    w['bdw'] = inp['dw_b'].reshape(C, 1).astype(np.float32)
    w['ln_g'] = inp['ln_g'].reshape(C, 1).astype(np.float32)
    w['ln_b'] = inp['ln_b'].reshape(C, 1).astype(np.float32)
    # offset/mask projections: wox/woy/wmk [256, 36] lhsT, col = g*9+p
    wox = np.zeros((C, 36), np.float32)
    woy = np.zeros((C, 36), np.float32)
    box = np.zeros((36, 1), np.float32)
    boy = np.zeros((36, 1), np.float32)
    ow, ob = np.asarray(inp['off_w'], np.float32), np.asarray(inp['off_b'], np.float32)
    for g in range(G):
        for p_ in range(P):
            wox[:, g * 9 + p_] = ow[g * 18 + p_ * 2 + 0]
            woy[:, g * 9 + p_] = ow[g * 18 + p_ * 2 + 1]
            box[g * 9 + p_, 0] = ob[g * 18 + p_ * 2 + 0]
            boy[g * 9 + p_, 0] = ob[g * 18 + p_ * 2 + 1]
    w['wox'], w['woy'], w['box'], w['boy'] = wox, woy, box, boy
    w['wmk'] = np.ascontiguousarray(inp['mask_w'].T).astype(np.float32)  # [256,36]
    import ml_dtypes as _mld
    for k in ('wox', 'woy', 'wmk'):
        w[k] = w[k].astype(_mld.bfloat16)
    w['bmk'] = inp['mask_b'].reshape(36, 1).astype(np.float32)
    w['wout'] = np.ascontiguousarray(inp['out_w'].T).astype(np.float32)  # [gc,o]
    w['bout'] = inp['out_b'].reshape(C, 1).astype(np.float32)
    Smats, E9, E9T = _host_consts()
    import ml_dtypes
    w['smats'] = np.ascontiguousarray(Smats.reshape(9, 36, 100)).astype(ml_dtypes.bfloat16)
    w['e9'], w['e9t'] = E9.astype(ml_dtypes.bfloat16), E9T
    e8 = np.zeros((8, 8, 128), np.float32)
    for n in range(8):
        e8[n, n, :] = 1.0
    w['e8sel'] = e8.reshape(8, 1024)
    return w


def _build(nc, tc, have_inb):
    import concourse.bass as bass
    import concourse.mybir as mybir
    from concourse.masks import make_identity
    f32 = mybir.dt.float32
    bf16 = mybir.dt.bfloat16
    AF = mybir.ActivationFunctionType
    ALU = mybir.AluOpType

    def dram(name, shape, dt=f32, kind="ExternalInput"):
        return nc.dram_tensor(name, shape, dt, kind=kind).ap()

    x_d = dram("x", [C_IN, S])
    wc_d = dram("wc", [C_IN, C])
    bc_d = dram("bc", [C, 1])
    win_d = dram("win", [C, C], bf16)
    dwd_d = dram("dwd", [128, 9, 2, 128], bf16)
    bdw_d = dram("bdw", [C, 1])
    lng_d = dram("lng", [C, 1])
    lnb_d = dram("lnb", [C, 1])
    wox_d = dram("wox", [C, 36], bf16)
    woy_d = dram("woy", [C, 36], bf16)
    wmk_d = dram("wmk", [C, 36], bf16)
    box_d = dram("box", [36, 1])
    boy_d = dram("boy", [36, 1])
    bmk_d = dram("bmk", [36, 1])
    wout_d = dram("wout", [C, C])
    bout_d = dram("bout", [C, 1])
    S_d = dram("smats", [9, 36, 100], bf16)
    e9_d = dram("e9", [36, 4], bf16)
    e9t_d = dram("e9t", [4, 36])
    e8_d = dram("e8sel", [8, 1024])
    inb_d = dram("inb", [1, C]) if have_inb else None
    out_d = dram("out", [C, S], kind="ExternalOutput")

    def load(pool, dr, shape, dt=f32, tag=None):
        t = pool.tile(shape, dt, tag=tag, name=tag)
        nc.sync.dma_start(out=t, in_=dr)
        return t

    def flat(t):
        return t.rearrange("p a b -> p (a b)")

    NB = 8          # n-blocks of 512
    CH = 4          # FMA oy-chunks
    CHH = H // CH   # 16 rows per chunk

    with tc.tile_pool(name="consts", bufs=1) as consts:
        wc = [load(consts, wc_d[0:128, :], [128, C], tag="wc0"),
              load(consts, wc_d[128:192, :], [64, C], tag="wc1")]
        bc = [load(consts, bc_d[0:128], [128, 1], tag="bc0"),
              load(consts, bc_d[128:256], [128, 1], tag="bc1")]
        win = [load(consts, win_d[0:128, :], [128, C], bf16, tag="win0"),
               load(consts, win_d[128:256, :], [128, C], bf16, tag="win1")]
        dwd = load(consts, dwd_d, [128, 9, 2, 128], bf16, tag="dwd")
        bdw = [load(consts, bdw_d[0:128], [128, 1], tag="bdw0"),
               load(consts, bdw_d[128:256], [128, 1], tag="bdw1")]
        lng = [load(consts, lng_d[0:128], [128, 1], tag="lng0"),
               load(consts, lng_d[128:256], [128, 1], tag="lng1")]
        lnb = [load(consts, lnb_d[0:128], [128, 1], tag="lnb0"),
               load(consts, lnb_d[128:256], [128, 1], tag="lnb1")]
        wox = [load(consts, wox_d[0:128, :], [128, 36], bf16, tag="wox0"),
               load(consts, wox_d[128:256, :], [128, 36], bf16, tag="wox1")]
        woy = [load(consts, woy_d[0:128, :], [128, 36], bf16, tag="woy0"),
               load(consts, woy_d[128:256, :], [128, 36], bf16, tag="woy1")]
        wmk = [load(consts, wmk_d[0:128, :], [128, 36], bf16, tag="wmk0"),
               load(consts, wmk_d[128:256, :], [128, 36], bf16, tag="wmk1")]
        box = load(consts, box_d, [36, 1], tag="box")
        boy = load(consts, boy_d, [36, 1], tag="boy")
        bmk = load(consts, bmk_d, [36, 1], tag="bmk")
        wout = [load(consts, wout_d[0:128, :], [128, C], tag="wout0"),
                load(consts, wout_d[128:256, :], [128, C], tag="wout1")]
        bout = [load(consts, bout_d[0:128], [128, 1], tag="bout0"),
                load(consts, bout_d[128:256], [128, 1], tag="bout1")]
        smt = [load(consts, S_d[i], [36, 100], bf16, tag=f"smt{i}") for i in range(9)]
        e9 = load(consts, e9_d, [36, 4], bf16, tag="e9")
        e9t = load(consts, e9t_d, [4, 36], tag="e9t")
        e8 = load(consts, e8_d, [8, 8, 128], tag="e8")
        ident = consts.tile([128, 128], f32, tag="ident", name="ident")
        make_identity(nc, ident)
        identb = consts.tile([128, 128], bf16, tag="identb", name="identb")
        make_identity(nc, identb)
        ones_k = consts.tile([128, 1], f32, tag="ones_k", name="ones_k")
        nc.vector.memset(ones_k, 1.0)
        eps8 = consts.tile([8, 1], f32, tag="eps8", name="eps8")
        nc.vector.memset(eps8, 1e-5)
        b_p1 = consts.tile([36, 1], f32, tag="b_p1", name="b_p1")
        nc.vector.memset(b_p1, 1.0)
        b_m1 = consts.tile([36, 1], f32, tag="b_m1", name="b_m1")
        nc.vector.memset(b_m1, -1.0)
        if have_inb:
            inb_b = consts.tile([128, C], f32, tag="inb", name="inb")
            nc.sync.dma_start(out=inb_b, in_=bass.AP(tensor=inb_d.tensor, offset=0,
                                                     ap=[[0, 128], [1, C]]))

        with tc.tile_pool(name="pers", bufs=1) as pers:
            # persistent mid-pipeline tensors
            t_ = [pers.tile([128, H, W], f32, tag=f"t{m}", name=f"t{m}") for m in range(2)]
            # val_T2: partition (h, ox), h = oy//32; free (oy%32, c)
            val_T = pers.tile([128, 32, C], bf16, tag="valT", name="valT")
            DWT = [pers.tile([128, H, 25], f32, tag=f"DWT{pr}", name=f"DWT{pr}")
                   for pr in range(2)]
            acc = [pers.tile([128, H, GC], f32, tag=f"acc{pr}", name=f"acc{pr}")
                   for pr in range(2)]

            with tc.tile_pool(name="psF", bufs=4, space="PSUM") as psF:
                with tc.tile_pool(name="M3", bufs=1) as M3:
                    DW = M3.tile([100, S], bf16, tag="DW", name="DW")
                    tbuf = M3.tile([128, 32, 100], f32, tag="tbuf", name="tbuf")

                    with tc.tile_pool(name="M1", bufs=1) as M1:
                        y = [M1.tile([128, H, W], bf16, tag=f"y{m}", name=f"y{m}")
                             for m in range(2)]
                        ypad = [M1.tile([128, 66, 66], bf16, tag=f"yp{m}", name=f"yp{m}")
                                for m in range(2)]

                        # ---- 1x1 conv (x streamed in 512-col slices) ----
                        with tc.tile_pool(name="xsP", bufs=3) as xsP:
                            for n in range(NB):
                                sl = slice(n * 512, (n + 1) * 512)
                                xs0 = load(xsP, x_d[0:128, sl], [128, 512], tag="xs0")
                                xs1 = load(xsP, x_d[128:192, sl], [64, 512], tag="xs1")
                                for mt in range(2):
                                    ps = psF.tile([128, 512], f32, tag="ps", name="ps")
                                    nc.tensor.matmul(ps, wc[0][:, mt * 128:(mt + 1) * 128], xs0, start=True, stop=False)
                                    nc.tensor.matmul(ps, wc[1][:, mt * 128:(mt + 1) * 128], xs1, start=False, stop=True)
                                    nc.scalar.activation(flat(y[mt])[:, sl], ps, AF.Identity, bias=bc[mt])

                        # ---- in_proj -> val_T2 (two oy-halves via psum col halves) ----
                        for oy in range(H):
                            h = oy // 32
                            ps = psF.tile([128, C], f32, tag="ps", name="ps")
                            po = ps[h * 64:(h + 1) * 64, :]
                            nc.tensor.matmul(po, y[0][:, oy, :], win[0], start=True, stop=False)
                            nc.tensor.matmul(po, y[1][:, oy, :], win[1], start=False, stop=True)
                            nc.scalar.activation(val_T[h * 64:(h + 1) * 64, oy % 32, :], po, AF.Identity)
                        if have_inb:
                            bcast = bass.AP(tensor=inb_b.tensor, offset=inb_b.offset,
                                            ap=[inb_b.ap[0], [0, 32], [1, C]])
                            nc.vector.tensor_add(val_T, val_T, bcast)

                        # ---- ypad + depthwise conv -> t ----
                        for mt in range(2):
                            nc.gpsimd.memset(ypad[mt], 0.0)
                            nc.vector.tensor_copy(ypad[mt][:, 1:65, 1:65], y[mt])
                        for mt in range(2):
                            for n in range(NB):
                                ps = psF.tile([128, 8, 64], f32, tag="ps", name="ps")
                                oy0 = n * 8
                                for tap in range(9):
                                    ky, kx = tap // 3, tap % 3
                                    nc.tensor.matmul(ps, dwd[:, tap, mt, :],
                                                     ypad[mt][:, oy0 + ky:oy0 + ky + 8, kx:kx + 64],
                                                     start=(tap == 0), stop=(tap == 8))
                                nc.scalar.activation(t_[mt][:, oy0:oy0 + 8, :], ps, AF.Identity, bias=bdw[mt])

                    # ---- M2: LN stats + normalize + offsets/masks + DW build ----
                    with tc.tile_pool(name="M2", bufs=1) as M2:
                        sA = M2.tile([8, 512], f32, tag="sA", name="sA")   # sum t -> mean -> mv
                        sB = M2.tile([8, 512], f32, tag="sB", name="sB")   # sum t2 -> var -> 1/var
                        sC = M2.tile([8, 512], f32, tag="sC", name="sC")   # mean^2
                        sD = M2.tile([8, 512], f32, tag="sD", name="sD")   # minv
                        with tc.tile_pool(name="sqP", bufs=3) as sqP:
                            for (isq, dst8) in ((0, sA), (1, sB)):
                                for n in range(NB):
                                    sl = slice(n * 512, (n + 1) * 512)
                                    ps = psF.tile([1, 512], f32, tag="ps", name="ps")
                                    if isq:
                                        for mt in range(2):
                                            tq = sqP.tile([128, 512], f32, tag="tq", name="tq")
                                            nc.scalar.activation(tq, flat(t_[mt])[:, sl], AF.Square)
                                            nc.tensor.matmul(ps, ones_k, tq, start=(mt == 0), stop=(mt == 1))
                                    else:
                                        nc.tensor.matmul(ps, ones_k, flat(t_[0])[:, sl], start=True, stop=False)
                                        nc.tensor.matmul(ps, ones_k, flat(t_[1])[:, sl], start=False, stop=True)
                                    stg = sqP.tile([1, 512], f32, tag="stg", name="stg")
                                    nc.vector.tensor_copy(stg, ps)
                                    nc.sync.dma_start(out=dst8[n:n + 1, :], in_=stg)
                        nc.scalar.mul(sA, sA, 1.0 / C)
                        nc.scalar.mul(sB, sB, 1.0 / C)
                        nc.scalar.activation(sC, sA, AF.Square)
                        nc.vector.scalar_tensor_tensor(sB, sC, -1.0, sB, op0=ALU.mult, op1=ALU.add)
                        nc.scalar.activation(sB, sB, AF.Identity, bias=eps8)
                        nc.vector.reciprocal(sB, sB)
                        nc.scalar.activation(sD, sB, AF.Sqrt)
                        nc.vector.tensor_mul(sA, sA, sD)

                        # normalize + gelu -> ta (bf16)
                        ta = [M2.tile([128, H, W], bf16, tag=f"ta{m}", name=f"ta{m}")
                              for m in range(2)]
                        with tc.tile_pool(name="uP", bufs=3) as uP:
                            for n in range(NB):
                                sl = slice(n * 512, (n + 1) * 512)
                                ps1 = psF.tile([128, 512], f32, tag="ps", name="ps")
                                ps2 = psF.tile([128, 512], f32, tag="ps", name="ps")
                                nc.tensor.matmul(ps1, e8[:, n, :], sD, start=True, stop=True)
                                nc.tensor.matmul(ps2, e8[:, n, :], sA, start=True, stop=True)
                                for mt in range(2):
                                    u = uP.tile([128, 512], f32, tag="u", name="u")
                                    nc.vector.tensor_mul(u, flat(t_[mt])[:, sl], ps1)
                                    nc.vector.tensor_sub(u, u, ps2)
                                    nc.scalar.activation(flat(ta[mt])[:, sl], u, AF.Gelu, bias=lnb[mt], scale=lng[mt])

                        # offsets/masks/hats/DW, s-chunked (4 chunks of 1024)
                        SC = 1024
                        for sc in range(4):
                            oxt = M2.tile([36, SC], bf16, tag="oxt", name="oxt")
                            oyt = M2.tile([36, SC], bf16, tag="oyt", name="oyt")
                            ex = M2.tile([36, SC], bf16, tag="ex", name="ex")
                            for nb2 in range(2):
                                n = sc * 2 + nb2
                                sl = slice(n * 512, (n + 1) * 512)
                                cl = slice(nb2 * 512, (nb2 + 1) * 512)
                                for (wgt, bia, dst2, fn) in ((wox, box, oxt, AF.Identity),
                                                             (woy, boy, oyt, AF.Identity),
                                                             (wmk, bmk, ex, AF.Exp)):
                                    ps = psF.tile([36, 512], f32, tag="ps", name="ps")
                                    nc.tensor.matmul(ps, wgt[0], flat(ta[0])[:, sl], start=True, stop=False)
                                    nc.tensor.matmul(ps, wgt[1], flat(ta[1])[:, sl], start=False, stop=True)
                                    nc.scalar.activation(dst2[:, cl], ps, fn, bias=bia)
                            rm = M2.tile([4, SC], f32, tag="rm", name="rm")
                            mask = M2.tile([36, SC], bf16, tag="mask", name="mask")
                            for nb2 in range(2):
                                cl = slice(nb2 * 512, (nb2 + 1) * 512)
                                ps = psF.tile([4, 512], f32, tag="ps", name="ps")
                                nc.tensor.matmul(ps, e9, ex[:, cl], start=True, stop=True)
                                nc.vector.reciprocal(rm[:, cl], ps)
                            for nb2 in range(2):
                                cl = slice(nb2 * 512, (nb2 + 1) * 512)
                                ps = psF.tile([36, 512], f32, tag="ps", name="ps")
                                nc.tensor.matmul(ps, e9t, rm[:, cl], start=True, stop=True)
                                nc.vector.tensor_mul(mask[:, cl], ex[:, cl], ps)

                            def hats(src2, pfx):
                                out3 = []
                                for (kk, off) in (("m", b_p1), ("c", None), ("p", b_m1)):
                                    ab = M2.tile([36, SC], bf16, tag="hab", name="hab")
                                    if off is None:
                                        nc.scalar.activation(ab, src2, AF.Abs)
                                    else:
                                        nc.scalar.activation(ab, src2, AF.Abs, bias=off)
                                    h = M2.tile([36, SC], bf16, tag=f"h{pfx}{kk}", name=f"h{pfx}{kk}")
                                    nc.scalar.activation(h, ab, AF.Relu, bias=b_p1, scale=-1.0)
                                    out3.append(h)
                                return out3
                            hx3 = hats(oxt, "x")
                            hy3 = hats(oyt, "y")
                            for yb in range(3):
                                nc.vector.tensor_mul(hy3[yb], mask, hy3[yb])  # hy -> m1 in place
                            psds = [psF.tile([100, 512], f32, tag=f"dwp{i}",
                                             name=f"dwp{i}", bufs=1) for i in range(2)]
                            for xb in range(3):
                                for yb in range(3):
                                    ki = xb * 3 + yb
                                    txb = M2.tile([36, SC], bf16, tag="txb", name="txb")
                                    nc.vector.tensor_mul(txb, hy3[yb], hx3[xb])
                                    for nb2 in range(2):
                                        cl = slice(nb2 * 512, (nb2 + 1) * 512)
                                        nc.tensor.matmul(psds[nb2], smt[ki], txb[:, cl],
                                                         start=(ki == 0), stop=(ki == 8))
                            for nb2 in range(2):
                                n = sc * 2 + nb2
                                nc.scalar.activation(DW[:, n * 512:(n + 1) * 512], psds[nb2], AF.Identity)

                    # ---- DW_T via PE transposes + remap DMA ----
                    for tch in range(32):
                        ps = psF.tile([128, 100], bf16, tag="ptr", name="ptr", bufs=2)
                        nc.tensor.transpose(ps, DW[:, tch * 128:(tch + 1) * 128], identb[0:100, 0:100])
                        nc.vector.tensor_copy(tbuf[:, tch, :], ps)
                    for pr in range(2):
                        for g2 in range(2):
                            g = pr * 2 + g2
                            for par in range(2):
                                d0 = DWT[pr][g2 * 64:(g2 + 1) * 64, :, :]
                                dst = bass.AP(tensor=d0.tensor, offset=d0.offset + par * 25,
                                              ap=[d0.ap[0], [50, 32], [1, 25]])
                                s0 = tbuf[par * 64:(par + 1) * 64, :, :]
                                src = bass.AP(tensor=s0.tensor, offset=s0.offset + g * 25,
                                              ap=[s0.ap[0], [100, 32], [1, 25]])
                                nc.sync.dma_start(out=dst, in_=src)

                # ---- FMA: 25-bin window sum, chunked over oy ----
                with tc.tile_pool(name="vxP", bufs=2) as vxP:
                    for ci in range(CH):
                        oy0 = ci * CHH
                        vxc = [[vxP.tile([128, CHH + 4, GC], bf16, tag=f"vx{pr}_{dxi}",
                                         name=f"vx{pr}_{dxi}")
                                for dxi in range(5)] for pr in range(2)]
                        for pr in range(2):
                            for dxi in range(5):
                                dx = dxi - 2
                                nc.gpsimd.memset(vxc[pr][dxi], 0.0)
                                # interior: vy = global val row; local iy idx = vy + 2 - oy0
                                vy_lo = max(0, oy0 - 2)
                                vy_hi = min(H, oy0 + CHH + 2)
                                for g2 in range(2):
                                    g = pr * 2 + g2
                                    lo = max(0, -dx)
                                    hi = min(64, 64 - dx)
                                    # split at the h-boundary (val row 32)
                                    for (a, b) in ((vy_lo, min(vy_hi, 32)), (max(vy_lo, 32), vy_hi)):
                                        if a >= b:
                                            continue
                                        h = a // 32
                                        dst = vxc[pr][dxi][g2 * 64 + lo:g2 * 64 + hi,
                                                           a + 2 - oy0:b + 2 - oy0, :]
                                        src = val_T[h * 64 + lo + dx:h * 64 + hi + dx,
                                                    a - h * 32:b - h * 32,
                                                    g * GC:(g + 1) * GC]
                                        nc.sync.dma_start(out=dst, in_=src)
                        for pr in range(2):
                            for oyl in range(CHH):
                                oy = oy0 + oyl
                                eng = nc.vector
                                first = True
                                for dyi in range(5):
                                    for dxi in range(5):
                                        d = dyi * 5 + dxi
                                        sc = DWT[pr][:, oy, d:d + 1]
                                        v = vxc[pr][dxi][:, oyl + dyi, :]
                                        o = acc[pr][:, oy, :]
                                        if first:
                                            eng.tensor_scalar_mul(o, v, sc)
                                            first = False
                                        else:
                                            eng.scalar_tensor_tensor(o, v, sc, o, op0=ALU.mult, op1=ALU.add)

            # ---- transpose acc back + out_proj ----
            with tc.tile_pool(name="psT", bufs=4, space="PSUM") as psT:
                with tc.tile_pool(name="E1", bufs=1) as E1:
                    RO = [E1.tile([128, H, W], f32, tag=f"ro{pr}", name=f"ro{pr}")
                          for pr in range(2)]
                    tb2 = E1.tile([128, 32, 128], f32, tag="tb2", name="tb2")
                    for pr in range(2):
                        for tch in range(32):
                            ps = psT.tile([128, 128], f32, tag="ps", name="ps")
                            nc.tensor.transpose(ps, flat(acc[pr])[:, tch * 128:(tch + 1) * 128], ident)
                            nc.scalar.activation(tb2[:, tch, :], ps, AF.Identity)
                        for g2 in range(2):
                            for par in range(2):
                                d0 = RO[pr][g2 * 64:(g2 + 1) * 64, :, :]
                                dst = bass.AP(tensor=d0.tensor, offset=d0.offset + par * 64,
                                              ap=[d0.ap[0], [128, 32], [1, 64]])
                                s0 = tb2[par * 64:(par + 1) * 64, :, :]
                                src = bass.AP(tensor=s0.tensor, offset=s0.offset + g2 * 64,
                                              ap=[s0.ap[0], [128, 32], [1, 64]])
                                nc.sync.dma_start(out=dst, in_=src)

                    for mt in range(2):
                        for n in range(NB):
                            sl = slice(n * 512, (n + 1) * 512)
                            ps = psT.tile([128, 512], f32, tag="ops", name="ops")
                            nc.tensor.matmul(ps, wout[0][:, mt * 128:(mt + 1) * 128],
                                             flat(RO[0])[:, sl], start=True, stop=False)
                            nc.tensor.matmul(ps, wout[1][:, mt * 128:(mt + 1) * 128],
                                             flat(RO[1])[:, sl], start=False, stop=True)
                            osb = E1.tile([128, 512], f32, tag="osb", name="osb", bufs=3)
                            nc.scalar.activation(osb, ps, AF.Identity, bias=bout[mt])
                            nc.sync.dma_start(out=out_d[mt * 128:(mt + 1) * 128, sl], in_=osb)


def _get_program(have_inb):
    key = ("prog", have_inb)
    if key not in _CACHE:
        import concourse.bacc as bacc
        import concourse.tile as tile
        nc = bacc.Bacc("TRN2", target_bir_lowering=False, debug=False,
                       enable_asserts=False)
        with tile.TileContext(nc) as tc:
            _build(nc, tc, have_inb)
        nc.compile()
        _CACHE[key] = nc
    return _CACHE[key]


def kernel(**inputs):
    inputs = {k: np.asarray(v) for k, v in inputs.items()}
    w = _prep_weights(inputs)
    have_inb = bool(np.any(w['inb']))
    nc = _get_program(have_inb)

    base = {
        'wc': w['wc'], 'bc': w['bc'], 'win': w['win'], 'dwd': w['dwd'],
        'bdw': w['bdw'], 'lng': w['ln_g'], 'lnb': w['ln_b'],
        'wox': w['wox'], 'woy': w['woy'], 'wmk': w['wmk'],
        'box': w['box'], 'boy': w['boy'], 'bmk': w['bmk'],
        'wout': w['wout'], 'bout': w['bout'],
        'smats': w['smats'], 'e9': w['e9'], 'e9t': w['e9t'], 'e8sel': w['e8sel'],
    }
    if have_inb:
        base['inb'] = w['inb'].reshape(1, C)
    x = np.asarray(inputs['x'], np.float32).reshape(N, C_IN, S)
    in_maps = []
    for core in range(NCORES):
        m = dict(base)
        m['x'] = np.ascontiguousarray(x[core])
        in_maps.append(m)

    from concourse import bass_utils
    res = bass_utils.run_bass_kernel_spmd(nc, in_maps, core_ids=list(range(NCORES)),
                                          trace=TRACE)
    global _LAST_EXEC_NS
    _LAST_EXEC_NS = res.exec_time_ns
    if TRACE:
        import sys
        print(f"[kernel] exec_time_ns={res.exec_time_ns} trace={res.instructions_and_trace[1] if res.instructions_and_trace else None}", file=sys.stderr)
    out = np.stack([r['out'].reshape(C, H, W) for r in res.results])
    return out.astype(np.float32)



# revision 7
# speedup vs baseline: 1.1665x; 1.1665x over previous
"""DCNv3_C Trainium2 Bass kernel.

8-core data parallelism over the batch (one image per NeuronCore).
Per core: 1x1 conv -> value proj -> depthwise 3x3 (block-diag matmuls)
-> LN+gelu -> offset/mask proj -> softmax -> dense 5x5 "hat" sampling
weights -> 25-bin weighted window sum (DVE scalar_tensor_tensor)
-> output proj.

DCNv3 bilinear sampling is rewritten exactly (for |offset|<=1) as a 5x5
locally-connected window:
  acc[s,g,c] = sum_{dy,dx in [-2,2]} DW[s,g,dy,dx] * VP[s+(dy,dx), g, c]
  DW[s,g,dy,dx] = sum_p mask_p * hat(gy_p+offy_p-dy) * hat(gx_p+offx_p-dx)
with hat(t)=max(0,1-|t|) and VP the value map zero-padded by 2.

v2: all matmul paths bf16/fp16 (x cast host-side), fp16 sampling
accumulator, LN rstd via ACT Rsqrt, softmax reciprocal on ACT, and the
whole back half (DW build -> transpose -> 25-bin FMA -> output-side
transpose -> out_proj -> store) pipelined in 4 row chunks so PE/ACT/DMA
work overlaps the DVE-bound FMA.
"""

import numpy as np

N, C_IN, C, H, W = 8, 192, 256, 64, 64
G, K, PAD = 4, 3, 1
GC = C // G          # 64
P = K * K            # 9
S = H * W            # 4096
NCORES = 8

_CACHE = {}
TRACE = False
_LAST_EXEC_NS = None


def _host_consts():
    # p = a*3+b with grid_x = a-1 (slowest), grid_y = b-1
    gx = np.repeat(np.arange(3) - 1, 3)
    gy = np.tile(np.arange(3) - 1, 3)
    # p-sum selection matrices, one per (xb, yb): [36, 100]
    # row (g, p) -> col g*25 + d, d = (dy+2)*5 + (dx+2)
    Smats = np.zeros((3, 3, 36, 100), np.float32)
    for xb in range(3):
        for yb in range(3):
            for g in range(G):
                for p_ in range(P):
                    dy = gy[p_] + yb - 1
                    dx = gx[p_] + xb - 1
                    d = (dy + 2) * 5 + (dx + 2)
                    Smats[xb, yb, g * 9 + p_, g * 25 + d] = 1.0
    E9 = np.zeros((36, 4), np.float32)     # per-group sums
    E9T = np.zeros((4, 36), np.float32)    # per-group broadcast
    for g in range(G):
        E9[g * 9:(g + 1) * 9, g] = 1.0
        E9T[g, g * 9:(g + 1) * 9] = 1.0
    return Smats, E9, E9T


def _prep_weights(inp):
    import ml_dtypes
    bf = ml_dtypes.bfloat16
    w = {}
    w['wc'] = np.ascontiguousarray(inp['conv_w'].T).astype(bf)            # [192,256]
    w['bc'] = inp['conv_b'].reshape(C, 1).astype(np.float32)
    w['win'] = np.ascontiguousarray(inp['in_w'].T).astype(bf)             # [c,o]
    w['inb'] = np.asarray(inp['in_b'], np.float32)
    # depthwise diag weights, partition-major: [128, 9, 2, 128]
    dwd = np.zeros((128, 9, 2, 128), np.float32)
    dw = inp['dw_w'].reshape(C, 9)
    for tap in range(9):
        for mt in range(2):
            for i in range(128):
                dwd[i, tap, mt, i] = dw[mt * 128 + i, tap]
    w['dwd'] = dwd.astype(bf)
    w['bdw'] = inp['dw_b'].reshape(C, 1).astype(np.float32)
    w['ln_g'] = inp['ln_g'].reshape(C, 1).astype(np.float32)
    w['ln_b'] = inp['ln_b'].reshape(C, 1).astype(np.float32)
    # offset/mask projections: wox/woy/wmk [256, 36] lhsT, col = g*9+p
    wox = np.zeros((C, 36), np.float32)
    woy = np.zeros((C, 36), np.float32)
    box = np.zeros((36, 1), np.float32)
    boy = np.zeros((36, 1), np.float32)
    ow, ob = np.asarray(inp['off_w'], np.float32), np.asarray(inp['off_b'], np.float32)
    for g in range(G):
        for p_ in range(P):
            wox[:, g * 9 + p_] = ow[g * 18 + p_ * 2 + 0]
            woy[:, g * 9 + p_] = ow[g * 18 + p_ * 2 + 1]
            box[g * 9 + p_, 0] = ob[g * 18 + p_ * 2 + 0]
            boy[g * 9 + p_, 0] = ob[g * 18 + p_ * 2 + 1]
    w['wox'], w['woy'] = wox.astype(bf), woy.astype(bf)
    w['box'], w['boy'] = box, boy
    w['wmk'] = np.ascontiguousarray(inp['mask_w'].T).astype(bf)           # [256,36]
    w['bmk'] = inp['mask_b'].reshape(36, 1).astype(np.float32)
    w['wout'] = np.ascontiguousarray(inp['out_w'].T).astype(np.float16)   # [gc,o]
    w['bout'] = inp['out_b'].reshape(C, 1).astype(np.float32)
    Smats, E9, E9T = _host_consts()
    w['smats'] = np.ascontiguousarray(Smats.reshape(9, 36, 100)).astype(bf)
    w['e9'] = E9.astype(bf)
    w['e9t'] = E9T.astype(bf)
    e8 = np.zeros((8, 8, 128), np.float32)
    for n in range(8):
        e8[n, n, :] = 1.0
    w['e8sel'] = e8.reshape(8, 1024).astype(bf)
    return w


def _build(nc, tc, have_inb):
    import concourse.bass as bass
    import concourse.mybir as mybir
    from concourse.masks import make_identity
    f32 = mybir.dt.float32
    bf16 = mybir.dt.bfloat16
    fp16 = mybir.dt.float16
    AF = mybir.ActivationFunctionType
    ALU = mybir.AluOpType

    def dram(name, shape, dt=f32, kind="ExternalInput"):
        return nc.dram_tensor(name, shape, dt, kind=kind).ap()

    x_d = dram("x", [C_IN, S], bf16)
    wc_d = dram("wc", [C_IN, C], bf16)
    bc_d = dram("bc", [C, 1])
    win_d = dram("win", [C, C], bf16)
    dwd_d = dram("dwd", [128, 9, 2, 128], bf16)
    bdw_d = dram("bdw", [C, 1])
    lng_d = dram("lng", [C, 1])
    lnb_d = dram("lnb", [C, 1])
    wox_d = dram("wox", [C, 36], bf16)
    woy_d = dram("woy", [C, 36], bf16)
    wmk_d = dram("wmk", [C, 36], bf16)
    box_d = dram("box", [36, 1])
    boy_d = dram("boy", [36, 1])
    bmk_d = dram("bmk", [36, 1])
    wout_d = dram("wout", [C, C], fp16)
    bout_d = dram("bout", [C, 1])
    S_d = dram("smats", [9, 36, 100], bf16)
    e9_d = dram("e9", [36, 4], bf16)
    e9t_d = dram("e9t", [4, 36], bf16)
    e8_d = dram("e8sel", [8, 1024], bf16)
    inb_d = dram("inb", [1, C]) if have_inb else None
    out_d = dram("out", [C, S], kind="ExternalOutput")

    def load(pool, dr, shape, dt=f32, tag=None):
        t = pool.tile(shape, dt, tag=tag, name=tag)
        nc.sync.dma_start(out=t, in_=dr)
        return t

    def flat(t):
        return t.rearrange("p a b -> p (a b)")

    NB = 8          # n-blocks of 512
    NCH = 4         # row chunks for the pipelined back half
    CHH = H // NCH  # 16 rows per chunk

    with tc.tile_pool(name="consts", bufs=1) as consts:
        wc = [load(consts, wc_d[0:128, :], [128, C], bf16, tag="wc0"),
              load(consts, wc_d[128:192, :], [64, C], bf16, tag="wc1")]
        bc = [load(consts, bc_d[0:128], [128, 1], tag="bc0"),
              load(consts, bc_d[128:256], [128, 1], tag="bc1")]
        win = [load(consts, win_d[0:128, :], [128, C], bf16, tag="win0"),
               load(consts, win_d[128:256, :], [128, C], bf16, tag="win1")]
        dwd = load(consts, dwd_d, [128, 9, 2, 128], bf16, tag="dwd")
        bdw = [load(consts, bdw_d[0:128], [128, 1], tag="bdw0"),
               load(consts, bdw_d[128:256], [128, 1], tag="bdw1")]
        lng = [load(consts, lng_d[0:128], [128, 1], tag="lng0"),
               load(consts, lng_d[128:256], [128, 1], tag="lng1")]
        lnb = [load(consts, lnb_d[0:128], [128, 1], tag="lnb0"),
               load(consts, lnb_d[128:256], [128, 1], tag="lnb1")]
        wox = [load(consts, wox_d[0:128, :], [128, 36], bf16, tag="wox0"),
               load(consts, wox_d[128:256, :], [128, 36], bf16, tag="wox1")]
        woy = [load(consts, woy_d[0:128, :], [128, 36], bf16, tag="woy0"),
               load(consts, woy_d[128:256, :], [128, 36], bf16, tag="woy1")]
        wmk = [load(consts, wmk_d[0:128, :], [128, 36], bf16, tag="wmk0"),
               load(consts, wmk_d[128:256, :], [128, 36], bf16, tag="wmk1")]
        box = load(consts, box_d, [36, 1], tag="box")
        boy = load(consts, boy_d, [36, 1], tag="boy")
        bmk = load(consts, bmk_d, [36, 1], tag="bmk")
        wout = [load(consts, wout_d[0:128, :], [128, C], fp16, tag="wout0"),
                load(consts, wout_d[128:256, :], [128, C], fp16, tag="wout1")]
        bout = [load(consts, bout_d[0:128], [128, 1], tag="bout0"),
                load(consts, bout_d[128:256], [128, 1], tag="bout1")]
        smt = [load(consts, S_d[i], [36, 100], bf16, tag=f"smt{i}") for i in range(9)]
        e9 = load(consts, e9_d, [36, 4], bf16, tag="e9")
        e9t = load(consts, e9t_d, [4, 36], bf16, tag="e9t")
        e8 = load(consts, e8_d, [8, 8, 128], bf16, tag="e8")
        identb = consts.tile([128, 128], bf16, tag="identb", name="identb")
        make_identity(nc, identb)
        identh = consts.tile([128, 128], fp16, tag="identh", name="identh")
        make_identity(nc, identh)
        ones_k = consts.tile([128, 1], bf16, tag="ones_k", name="ones_k")
        nc.vector.memset(ones_k, 1.0)
        eps8 = consts.tile([8, 1], f32, tag="eps8", name="eps8")
        nc.vector.memset(eps8, 1e-5)
        b_p1 = consts.tile([36, 1], f32, tag="b_p1", name="b_p1")
        nc.vector.memset(b_p1, 1.0)
        b_m1 = consts.tile([36, 1], f32, tag="b_m1", name="b_m1")
        nc.vector.memset(b_m1, -1.0)
        if have_inb:
            inb_b = consts.tile([128, C], f32, tag="inb", name="inb")
            nc.sync.dma_start(out=inb_b, in_=bass.AP(tensor=inb_d.tensor, offset=0,
                                                     ap=[[0, 128], [1, C]]))

        with tc.tile_pool(name="pers", bufs=1) as pers:
            # persistent mid-pipeline tensors
            # val_T: partition (h, ox), h = oy//32; free (oy%32, c)  (fp16)
            val_T = pers.tile([128, 32, C], fp16, tag="valT", name="valT")
            DWT = [pers.tile([128, H, 25], f32, tag=f"DWT{pr}", name=f"DWT{pr}")
                   for pr in range(2)]
            acc = [pers.tile([128, H, GC], fp16, tag=f"acc{pr}", name=f"acc{pr}")
                   for pr in range(2)]

            with tc.tile_pool(name="psF", bufs=2, space="PSUM") as psF:
                with tc.tile_pool(name="M3", bufs=1) as M3:
                    DW = M3.tile([100, S], fp16, tag="DW", name="DW")
                    t_ = [M3.tile([128, H, W], bf16, tag=f"t{m}", name=f"t{m}")
                          for m in range(2)]

                    with tc.tile_pool(name="M1", bufs=1) as M1:
                        y = [M1.tile([128, H, W], bf16, tag=f"y{m}", name=f"y{m}")
                             for m in range(2)]
                        ypad = [M1.tile([128, 66, 66], bf16, tag=f"yp{m}", name=f"yp{m}")
                                for m in range(2)]

                        # ---- 1x1 conv (x streamed in 512-col slices, bf16) ----
                        with tc.tile_pool(name="xsP", bufs=3) as xsP:
                            for n in range(NB):
                                sl = slice(n * 512, (n + 1) * 512)
                                xs0 = load(xsP, x_d[0:128, sl], [128, 512], bf16, tag="xs0")
                                xs1 = load(xsP, x_d[128:192, sl], [64, 512], bf16, tag="xs1")
                                for mt in range(2):
                                    ps = psF.tile([128, 512], f32, tag="ps", name="ps")
                                    nc.tensor.matmul(ps, wc[0][:, mt * 128:(mt + 1) * 128], xs0, start=True, stop=False)
                                    nc.tensor.matmul(ps, wc[1][:, mt * 128:(mt + 1) * 128], xs1, start=False, stop=True)
                                    nc.scalar.activation(flat(y[mt])[:, sl], ps, AF.Identity, bias=bc[mt])

                        # ---- in_proj -> val_T (fp16, two oy-halves via psum halves) ----
                        for oy in range(H):
                            h = oy // 32
                            ps = psF.tile([128, C], f32, tag="ps", name="ps")
                            po = ps[h * 64:(h + 1) * 64, :]
                            nc.tensor.matmul(po, y[0][:, oy, :], win[0], start=True, stop=False)
                            nc.tensor.matmul(po, y[1][:, oy, :], win[1], start=False, stop=True)
                            nc.scalar.activation(val_T[h * 64:(h + 1) * 64, oy % 32, :], po, AF.Identity)
                        if have_inb:
                            bcast = bass.AP(tensor=inb_b.tensor, offset=inb_b.offset,
                                            ap=[inb_b.ap[0], [0, 32], [1, C]])
                            nc.vector.tensor_add(val_T, val_T, bcast)

                        # ---- ypad + depthwise conv -> t (bf16) ----
                        for mt in range(2):
                            nc.gpsimd.memset(ypad[mt], 0.0)
                            nc.vector.tensor_copy(ypad[mt][:, 1:65, 1:65], y[mt])
                        for mt in range(2):
                            for n in range(NB):
                                ps = psF.tile([128, 8, 64], f32, tag="ps", name="ps")
                                oy0 = n * 8
                                for tap in range(9):
                                    ky, kx = tap // 3, tap % 3
                                    nc.tensor.matmul(ps, dwd[:, tap, mt, :],
                                                     ypad[mt][:, oy0 + ky:oy0 + ky + 8, kx:kx + 64],
                                                     start=(tap == 0), stop=(tap == 8))
                                nc.scalar.activation(t_[mt][:, oy0:oy0 + 8, :], ps, AF.Identity, bias=bdw[mt])

                    # ---- M2: LN stats + normalize + offsets/masks + DW/FMA pipeline ----
                    with tc.tile_pool(name="M2", bufs=1) as M2:
                        sA = M2.tile([8, 512], f32, tag="sA", name="sA")   # mean -> mean*rstd
                        sB = M2.tile([8, 512], f32, tag="sB", name="sB")   # E[t^2] -> var
                        sD = M2.tile([8, 512], f32, tag="sD", name="sD")   # mean^2 -> rstd
                        sC = sD
                        sDb = M2.tile([8, 512], bf16, tag="sDb", name="sDb")
                        sAb = M2.tile([8, 512], bf16, tag="sAb", name="sAb")
                        with tc.tile_pool(name="sqP", bufs=3) as sqP:
                            for (isq, dst8) in ((0, sA), (1, sB)):
                                for n in range(NB):
                                    sl = slice(n * 512, (n + 1) * 512)
                                    ps = psF.tile([1, 512], f32, tag="ps", name="ps")
                                    if isq:
                                        for mt in range(2):
                                            tq = sqP.tile([128, 512], bf16, tag="tq", name="tq")
                                            nc.scalar.activation(tq, flat(t_[mt])[:, sl], AF.Square)
                                            nc.tensor.matmul(ps, ones_k, tq, start=(mt == 0), stop=(mt == 1))
                                    else:
                                        nc.tensor.matmul(ps, ones_k, flat(t_[0])[:, sl], start=True, stop=False)
                                        nc.tensor.matmul(ps, ones_k, flat(t_[1])[:, sl], start=False, stop=True)
                                    stg = sqP.tile([1, 512], f32, tag="stg", name="stg")
                                    nc.vector.tensor_copy(stg, ps)
                                    nc.sync.dma_start(out=dst8[n:n + 1, :], in_=stg)
                        nc.scalar.mul(sA, sA, 1.0 / C)
                        nc.scalar.mul(sB, sB, 1.0 / C)
                        nc.scalar.activation(sC, sA, AF.Square)
                        nc.vector.scalar_tensor_tensor(sB, sC, -1.0, sB, op0=ALU.mult, op1=ALU.add)
                        nc.scalar.activation(sB, sB, AF.Identity, bias=eps8)
                        nc.vector.reciprocal(sB, sB)
                        nc.scalar.activation(sD, sB, AF.Sqrt)
                        nc.vector.tensor_mul(sA, sA, sD)
                        nc.vector.tensor_copy(sDb, sD)
                        nc.vector.tensor_copy(sAb, sA)

                        # normalize + gelu -> in-place into t_ (bf16)
                        ta = t_
                        with tc.tile_pool(name="uP", bufs=3) as uP:
                            for n in range(NB):
                                sl = slice(n * 512, (n + 1) * 512)
                                ps1 = psF.tile([128, 512], f32, tag="ps", name="ps")
                                ps2 = psF.tile([128, 512], f32, tag="ps", name="ps")
                                nc.tensor.matmul(ps1, e8[:, n, :], sDb, start=True, stop=True)
                                nc.tensor.matmul(ps2, e8[:, n, :], sAb, start=True, stop=True)
                                for mt in range(2):
                                    u = uP.tile([128, 512], f32, tag="u", name="u")
                                    nc.vector.tensor_mul(u, flat(t_[mt])[:, sl], ps1)
                                    nc.vector.tensor_sub(u, u, ps2)
                                    nc.scalar.activation(flat(ta[mt])[:, sl], u, AF.Gelu, bias=lnb[mt], scale=lng[mt])

                        # ---- chunked pipeline: offsets/masks/DW -> DWT -> FMA
                        #      -> transpose-out -> out_proj, per 16-row chunk ----
                        SC = 1024
                        with tc.tile_pool(name="vxP", bufs=1) as vxP, \
                             tc.tile_pool(name="tbP", bufs=2) as tbP, \
                             tc.tile_pool(name="E1", bufs=2) as E1, \
                             tc.tile_pool(name="psT", bufs=2, space="PSUM") as psT:
                            # vxc buffers persist across chunks (bufs=2 alternate);
                            # only interior rows get rewritten each chunk, edge
                            # zeros from the initial memset persist.
                            vxc_bufs = []
                            for bi in range(2):
                                vb = [[vxP.tile([128, CHH + 4, GC], fp16,
                                                tag=f"vx{bi}_{pr}_{dxi}",
                                                name=f"vx{bi}_{pr}_{dxi}")
                                       for dxi in range(5)] for pr in range(2)]
                                for pr in range(2):
                                    for dxi in range(5):
                                        nc.gpsimd.memset(vb[pr][dxi], 0.0)
                                vxc_bufs.append(vb)

                            for ci in range(NCH):
                                oy0 = ci * CHH
                                sl_c = slice(ci * SC, (ci + 1) * SC)
                                # --- offsets / masks / hats / DW for this chunk ---
                                oxt = M2.tile([36, SC], bf16, tag="oxt", name="oxt")
                                oyt = M2.tile([36, SC], bf16, tag="oyt", name="oyt")
                                ex = M2.tile([36, SC], bf16, tag="ex", name="ex")
                                for nb2 in range(2):
                                    n = ci * 2 + nb2
                                    sl = slice(n * 512, (n + 1) * 512)
                                    cl = slice(nb2 * 512, (nb2 + 1) * 512)
                                    for (wgt, bia, dst2, fn) in ((wox, box, oxt, AF.Identity),
                                                                 (woy, boy, oyt, AF.Identity),
                                                                 (wmk, bmk, ex, AF.Exp)):
                                        ps = psF.tile([36, 512], f32, tag="ps", name="ps")
                                        nc.tensor.matmul(ps, wgt[0], flat(ta[0])[:, sl], start=True, stop=False)
                                        nc.tensor.matmul(ps, wgt[1], flat(ta[1])[:, sl], start=False, stop=True)
                                        nc.scalar.activation(dst2[:, cl], ps, fn, bias=bia)
                                rm = M2.tile([4, SC], bf16, tag="rm", name="rm")
                                mask = M2.tile([36, SC], bf16, tag="mask", name="mask")
                                for nb2 in range(2):
                                    cl = slice(nb2 * 512, (nb2 + 1) * 512)
                                    ps = psF.tile([4, 512], f32, tag="ps", name="ps")
                                    nc.tensor.matmul(ps, e9, ex[:, cl], start=True, stop=True)
                                    rmf = M2.tile([4, 512], f32, tag="rmf", name="rmf")
                                    nc.vector.reciprocal(rmf, ps)
                                    nc.vector.tensor_copy(rm[:, cl], rmf)
                                for nb2 in range(2):
                                    cl = slice(nb2 * 512, (nb2 + 1) * 512)
                                    ps = psF.tile([36, 512], f32, tag="ps", name="ps")
                                    nc.tensor.matmul(ps, e9t, rm[:, cl], start=True, stop=True)
                                    nc.vector.tensor_mul(mask[:, cl], ex[:, cl], ps)

                                def hats(src2, pfx):
                                    out3 = []
                                    for (kk, off) in (("m", b_p1), ("c", None), ("p", b_m1)):
                                        ab = M2.tile([36, SC], bf16, tag="hab", name="hab")
                                        if off is None:
                                            nc.scalar.activation(ab, src2, AF.Abs)
                                        else:
                                            nc.scalar.activation(ab, src2, AF.Abs, bias=off)
                                        h = M2.tile([36, SC], bf16, tag=f"h{pfx}{kk}", name=f"h{pfx}{kk}")
                                        nc.scalar.activation(h, ab, AF.Relu, bias=b_p1, scale=-1.0)
                                        out3.append(h)
                                    return out3
                                hx3 = hats(oxt, "x")
                                hy3 = hats(oyt, "y")
                                for yb in range(3):
                                    nc.vector.tensor_mul(hy3[yb], mask, hy3[yb])  # hy -> m*hy
                                psds = [psF.tile([100, 512], f32, tag=f"dwp{i}",
                                                 name=f"dwp{i}", bufs=1) for i in range(2)]
                                for xb in range(3):
                                    for yb in range(3):
                                        ki = xb * 3 + yb
                                        txb = M2.tile([36, SC], bf16, tag="txb", name="txb")
                                        nc.vector.tensor_mul(txb, hy3[yb], hx3[xb])
                                        for nb2 in range(2):
                                            cl = slice(nb2 * 512, (nb2 + 1) * 512)
                                            nc.tensor.matmul(psds[nb2], smt[ki], txb[:, cl],
                                                             start=(ki == 0), stop=(ki == 8))
                                for nb2 in range(2):
                                    n = ci * 2 + nb2
                                    nc.scalar.activation(DW[:, n * 512:(n + 1) * 512], psds[nb2], AF.Identity)

                                # --- DW chunk -> DWT via PE transposes + remap DMA ---
                                tbuf = tbP.tile([128, 8, 100], f32, tag="tbuf", name="tbuf")
                                for tch in range(8):
                                    gch = ci * 8 + tch
                                    ps = psT.tile([128, 128], fp16, tag="tr", name="tr")
                                    nc.tensor.transpose(ps[:, 0:100], DW[:, gch * 128:(gch + 1) * 128], identh[0:100, 0:100])
                                    nc.vector.tensor_copy(tbuf[:, tch, :], ps[:, 0:100])
                                for pr in range(2):
                                    for g2 in range(2):
                                        g = pr * 2 + g2
                                        for par in range(2):
                                            d0 = DWT[pr][g2 * 64:(g2 + 1) * 64, :, :]
                                            dst = bass.AP(tensor=d0.tensor,
                                                          offset=d0.offset + (oy0 + par) * 25,
                                                          ap=[d0.ap[0], [50, 8], [1, 25]])
                                            s0 = tbuf[par * 64:(par + 1) * 64, :, :]
                                            src = bass.AP(tensor=s0.tensor, offset=s0.offset + g * 25,
                                                          ap=[s0.ap[0], [100, 8], [1, 25]])
                                            nc.sync.dma_start(out=dst, in_=src)

                                # --- FMA chunk: load shifted value slices, 25-bin STT ---
                                vxc = vxc_bufs[ci % 2]
                                vy_lo = max(0, oy0 - 2)
                                vy_hi = min(H, oy0 + CHH + 2)
                                for pr in range(2):
                                    for dxi in range(5):
                                        dx = dxi - 2
                                        for g2 in range(2):
                                            g = pr * 2 + g2
                                            lo = max(0, -dx)
                                            hi = min(64, 64 - dx)
                                            for (a, b) in ((vy_lo, min(vy_hi, 32)), (max(vy_lo, 32), vy_hi)):
                                                if a >= b:
                                                    continue
                                                h = a // 32
                                                dst = vxc[pr][dxi][g2 * 64 + lo:g2 * 64 + hi,
                                                                   a + 2 - oy0:b + 2 - oy0, :]
                                                src = val_T[h * 64 + lo + dx:h * 64 + hi + dx,
                                                            a - h * 32:b - h * 32,
                                                            g * GC:(g + 1) * GC]
                                                nc.sync.dma_start(out=dst, in_=src)
                                        # zero rows outside the copied band (stale data
                                        # from the other chunk sharing this buffer)
                                        if vy_lo > oy0 - 2:
                                            nc.gpsimd.memset(vxc[pr][dxi][:, 0:vy_lo - (oy0 - 2), :], 0.0)
                                        if vy_hi < oy0 + CHH + 2:
                                            nc.gpsimd.memset(
                                                vxc[pr][dxi][:, vy_hi - (oy0 - 2):CHH + 4, :], 0.0)
                                for pr in range(2):
                                    for oyl in range(CHH):
                                        oy = oy0 + oyl
                                        eng = nc.vector
                                        first = True
                                        for dyi in range(5):
                                            for dxi in range(5):
                                                d = dyi * 5 + dxi
                                                sc = DWT[pr][:, oy, d:d + 1]
                                                v = vxc[pr][dxi][:, oyl + dyi, :]
                                                o = acc[pr][:, oy, :]
                                                if first:
                                                    eng.tensor_scalar_mul(o, v, sc)
                                                    first = False
                                                else:
                                                    eng.scalar_tensor_tensor(o, v, sc, o, op0=ALU.mult, op1=ALU.add)

                                # --- transpose acc chunk back + out_proj + store ---
                                RO = [E1.tile([128, CHH, W], fp16, tag=f"ro{pr}", name=f"ro{pr}")
                                      for pr in range(2)]
                                tb2 = E1.tile([128, 8, 128], fp16, tag="tb2", name="tb2")
                                for pr in range(2):
                                    for tch in range(8):
                                        ps = psT.tile([128, 128], fp16, tag="tr", name="tr")
                                        nc.tensor.transpose(ps, flat(acc[pr])[:, ci * SC + tch * 128:ci * SC + (tch + 1) * 128], identh)
                                        nc.scalar.activation(tb2[:, tch, :], ps, AF.Identity)
                                    for g2 in range(2):
                                        for par in range(2):
                                            d0 = RO[pr][g2 * 64:(g2 + 1) * 64, :, :]
                                            dst = bass.AP(tensor=d0.tensor, offset=d0.offset + par * 64,
                                                          ap=[d0.ap[0], [128, 8], [1, 64]])
                                            s0 = tb2[par * 64:(par + 1) * 64, :, :]
                                            src = bass.AP(tensor=s0.tensor, offset=s0.offset + g2 * 64,
                                                          ap=[s0.ap[0], [128, 8], [1, 64]])
                                            nc.sync.dma_start(out=dst, in_=src)

                                for mt in range(2):
                                    for n2 in range(2):
                                        sl = slice(ci * SC + n2 * 512, ci * SC + (n2 + 1) * 512)
                                        cl = slice(n2 * 512, (n2 + 1) * 512)
                                        ps = psF.tile([128, 512], f32, tag="ops", name="ops")
                                        nc.tensor.matmul(ps, wout[0][:, mt * 128:(mt + 1) * 128],
                                                         flat(RO[0])[:, cl], start=True, stop=False)
                                        nc.tensor.matmul(ps, wout[1][:, mt * 128:(mt + 1) * 128],
                                                         flat(RO[1])[:, cl], start=False, stop=True)
                                        osb = E1.tile([128, 512], f32, tag="osb", name="osb", bufs=2)
                                        nc.scalar.activation(osb, ps, AF.Identity, bias=bout[mt])
                                        nc.sync.dma_start(out=out_d[mt * 128:(mt + 1) * 128, sl], in_=osb)


def _get_program(have_inb):
    key = ("prog", have_inb)
    if key not in _CACHE:
        import concourse.bacc as bacc
        import concourse.tile as tile
        nc = bacc.Bacc("TRN2", target_bir_lowering=False, debug=False,
                       enable_asserts=False)
        with tile.TileContext(nc) as tc:
            _build(nc, tc, have_inb)
        nc.compile()
        _CACHE[key] = nc
    return _CACHE[key]


def kernel(**inputs):
    import ml_dtypes
    inputs = {k: np.asarray(v) for k, v in inputs.items()}
    w = _prep_weights(inputs)
    have_inb = bool(np.any(w['inb']))
    nc = _get_program(have_inb)

    base = {
        'wc': w['wc'], 'bc': w['bc'], 'win': w['win'], 'dwd': w['dwd'],
        'bdw': w['bdw'], 'lng': w['ln_g'], 'lnb': w['ln_b'],
        'wox': w['wox'], 'woy': w['woy'], 'wmk': w['wmk'],
        'box': w['box'], 'boy': w['boy'], 'bmk': w['bmk'],
        'wout': w['wout'], 'bout': w['bout'],
        'smats': w['smats'], 'e9': w['e9'], 'e9t': w['e9t'], 'e8sel': w['e8sel'],
    }
    if have_inb:
        base['inb'] = w['inb'].reshape(1, C)
    x = np.asarray(inputs['x'], np.float32).reshape(N, C_IN, S).astype(ml_dtypes.bfloat16)
    in_maps = []
    for core in range(NCORES):
        m = dict(base)
        m['x'] = np.ascontiguousarray(x[core])
        in_maps.append(m)

    from concourse import bass_utils
    res = bass_utils.run_bass_kernel_spmd(nc, in_maps, core_ids=list(range(NCORES)),
                                          trace=TRACE)
    global _LAST_EXEC_NS
    _LAST_EXEC_NS = res.exec_time_ns
    if TRACE:
        import sys
        print(f"[kernel] exec_time_ns={res.exec_time_ns} trace={res.instructions_and_trace[1] if res.instructions_and_trace else None}", file=sys.stderr)
    out = np.stack([r['out'].reshape(C, H, W) for r in res.results])
    return out.astype(np.float32)


# revision 8
# speedup vs baseline: 1.6055x; 1.3764x over previous
"""DCNv3_C Trainium2 Bass kernel.

8-core data parallelism over the batch (one image per NeuronCore).
Per core: 1x1 conv -> value proj -> depthwise 3x3 (block-diag matmuls)
-> LN+gelu -> offset/mask proj -> softmax -> dense 5x5 "hat" sampling
weights -> 25-bin weighted window sum (DVE scalar_tensor_tensor)
-> output proj.

DCNv3 bilinear sampling is rewritten exactly (for |offset|<=1) as a 5x5
locally-connected window:
  acc[s,g,c] = sum_{dy,dx in [-2,2]} DW[s,g,dy,dx] * VP[s+(dy,dx), g, c]
  DW[s,g,dy,dx] = sum_p mask_p * hat(gy_p+offy_p-dy) * hat(gx_p+offx_p-dx)
with hat(t)=max(0,1-|t|) and VP the value map zero-padded by 2.

v2: all matmul paths bf16/fp16 (x cast host-side), fp16 sampling
accumulator, LN rstd via ACT Rsqrt, softmax reciprocal on ACT, and the
whole back half (DW build -> transpose -> 25-bin FMA -> output-side
transpose -> out_proj -> store) pipelined in 4 row chunks so PE/ACT/DMA
work overlaps the DVE-bound FMA.
"""

import numpy as np

N, C_IN, C, H, W = 8, 192, 256, 64, 64
G, K, PAD = 4, 3, 1
GC = C // G          # 64
P = K * K            # 9
S = H * W            # 4096
NCORES = 8

_CACHE = {}
TRACE = False
_LAST_EXEC_NS = None


def _host_consts():
    # p = a*3+b with grid_x = a-1 (slowest), grid_y = b-1
    gx = np.repeat(np.arange(3) - 1, 3)
    gy = np.tile(np.arange(3) - 1, 3)
    # p-sum selection matrices, one per (xb, yb): [36, 100]
    # row (g, p) -> col g*25 + d, d = (dy+2)*5 + (dx+2)
    Smats = np.zeros((3, 3, 36, 100), np.float32)
    for xb in range(3):
        for yb in range(3):
            for g in range(G):
                for p_ in range(P):
                    dy = gy[p_] + yb - 1
                    dx = gx[p_] + xb - 1
                    d = (dy + 2) * 5 + (dx + 2)
                    Smats[xb, yb, g * 9 + p_, g * 25 + d] = 1.0
    E9 = np.zeros((36, 4), np.float32)     # per-group sums
    E9T = np.zeros((4, 36), np.float32)    # per-group broadcast
    for g in range(G):
        E9[g * 9:(g + 1) * 9, g] = 1.0
        E9T[g, g * 9:(g + 1) * 9] = 1.0
    return Smats, E9, E9T


def _prep_weights(inp):
    import ml_dtypes
    bf = ml_dtypes.bfloat16
    w = {}
    w['wc'] = np.ascontiguousarray(inp['conv_w'].T).astype(bf)            # [192,256]
    w['bc'] = inp['conv_b'].reshape(C, 1).astype(np.float32)
    w['win'] = np.ascontiguousarray(inp['in_w'].T).astype(bf)             # [c,o]
    w['inb'] = np.asarray(inp['in_b'], np.float32)
    # depthwise diag weights, partition-major: [128, 9, 2, 128]
    dwd = np.zeros((128, 9, 2, 128), np.float32)
    dw = inp['dw_w'].reshape(C, 9)
    for tap in range(9):
        for mt in range(2):
            for i in range(128):
                dwd[i, tap, mt, i] = dw[mt * 128 + i, tap]
    w['dwd'] = dwd.astype(bf)
    w['bdw'] = inp['dw_b'].reshape(C, 1).astype(np.float32)
    w['ln_g'] = inp['ln_g'].reshape(C, 1).astype(np.float32)
    w['ln_b'] = inp['ln_b'].reshape(C, 1).astype(np.float32)
    # offset/mask projections: wox/woy/wmk [256, 36] lhsT, col = g*9+p
    wox = np.zeros((C, 36), np.float32)
    woy = np.zeros((C, 36), np.float32)
    box = np.zeros((36, 1), np.float32)
    boy = np.zeros((36, 1), np.float32)
    ow, ob = np.asarray(inp['off_w'], np.float32), np.asarray(inp['off_b'], np.float32)
    for g in range(G):
        for p_ in range(P):
            wox[:, g * 9 + p_] = ow[g * 18 + p_ * 2 + 0]
            woy[:, g * 9 + p_] = ow[g * 18 + p_ * 2 + 1]
            box[g * 9 + p_, 0] = ob[g * 18 + p_ * 2 + 0]
            boy[g * 9 + p_, 0] = ob[g * 18 + p_ * 2 + 1]
    w['wox'], w['woy'] = wox.astype(bf), woy.astype(bf)
    w['box'], w['boy'] = box, boy
    w['wmk'] = np.ascontiguousarray(inp['mask_w'].T).astype(bf)           # [256,36]
    w['bmk'] = inp['mask_b'].reshape(36, 1).astype(np.float32)
    w['wout'] = np.ascontiguousarray(inp['out_w'].T).astype(np.float16)   # [gc,o]
    w['bout'] = inp['out_b'].reshape(C, 1).astype(np.float32)
    Smats, E9, E9T = _host_consts()
    w['smats'] = np.ascontiguousarray(Smats.reshape(9, 36, 100)).astype(bf)
    w['e9'] = E9.astype(bf)
    w['e9t'] = E9T.astype(bf)
    e8 = np.zeros((8, 8, 128), np.float32)
    for n in range(8):
        e8[n, n, :] = 1.0
    w['e8sel'] = e8.reshape(8, 1024).astype(bf)
    return w


def _build(nc, tc, have_inb):
    import concourse.bass as bass
    import concourse.mybir as mybir
    from concourse.masks import make_identity
    f32 = mybir.dt.float32
    bf16 = mybir.dt.bfloat16
    fp16 = mybir.dt.float16
    AF = mybir.ActivationFunctionType
    ALU = mybir.AluOpType

    def dram(name, shape, dt=f32, kind="ExternalInput"):
        return nc.dram_tensor(name, shape, dt, kind=kind).ap()

    x_d = dram("x", [C_IN, S], bf16)
    wc_d = dram("wc", [C_IN, C], bf16)
    bc_d = dram("bc", [C, 1])
    win_d = dram("win", [C, C], bf16)
    dwd_d = dram("dwd", [128, 9, 2, 128], bf16)
    bdw_d = dram("bdw", [C, 1])
    lng_d = dram("lng", [C, 1])
    lnb_d = dram("lnb", [C, 1])
    wox_d = dram("wox", [C, 36], bf16)
    woy_d = dram("woy", [C, 36], bf16)
    wmk_d = dram("wmk", [C, 36], bf16)
    box_d = dram("box", [36, 1])
    boy_d = dram("boy", [36, 1])
    bmk_d = dram("bmk", [36, 1])
    wout_d = dram("wout", [C, C], fp16)
    bout_d = dram("bout", [C, 1])
    S_d = dram("smats", [9, 36, 100], bf16)
    e9_d = dram("e9", [36, 4], bf16)
    e9t_d = dram("e9t", [4, 36], bf16)
    e8_d = dram("e8sel", [8, 1024], bf16)
    inb_d = dram("inb", [1, C]) if have_inb else None
    out_d = dram("out", [C, S], kind="ExternalOutput")

    def load(pool, dr, shape, dt=f32, tag=None):
        t = pool.tile(shape, dt, tag=tag, name=tag)
        nc.sync.dma_start(out=t, in_=dr)
        return t

    def flat(t):
        return t.rearrange("p a b -> p (a b)")

    NB = 8          # n-blocks of 512
    NCH = 4         # row chunks for the pipelined back half
    CHH = H // NCH  # 16 rows per chunk

    with tc.tile_pool(name="consts", bufs=1) as consts:
        wc = [load(consts, wc_d[0:128, :], [128, C], bf16, tag="wc0"),
              load(consts, wc_d[128:192, :], [64, C], bf16, tag="wc1")]
        bc = [load(consts, bc_d[0:128], [128, 1], tag="bc0"),
              load(consts, bc_d[128:256], [128, 1], tag="bc1")]
        win = [load(consts, win_d[0:128, :], [128, C], bf16, tag="win0"),
               load(consts, win_d[128:256, :], [128, C], bf16, tag="win1")]
        dwd = load(consts, dwd_d, [128, 9, 2, 128], bf16, tag="dwd")
        bdw = [load(consts, bdw_d[0:128], [128, 1], tag="bdw0"),
               load(consts, bdw_d[128:256], [128, 1], tag="bdw1")]
        lng = [load(consts, lng_d[0:128], [128, 1], tag="lng0"),
               load(consts, lng_d[128:256], [128, 1], tag="lng1")]
        lnb = [load(consts, lnb_d[0:128], [128, 1], tag="lnb0"),
               load(consts, lnb_d[128:256], [128, 1], tag="lnb1")]
        wox = [load(consts, wox_d[0:128, :], [128, 36], bf16, tag="wox0"),
               load(consts, wox_d[128:256, :], [128, 36], bf16, tag="wox1")]
        woy = [load(consts, woy_d[0:128, :], [128, 36], bf16, tag="woy0"),
               load(consts, woy_d[128:256, :], [128, 36], bf16, tag="woy1")]
        wmk = [load(consts, wmk_d[0:128, :], [128, 36], bf16, tag="wmk0"),
               load(consts, wmk_d[128:256, :], [128, 36], bf16, tag="wmk1")]
        box = load(consts, box_d, [36, 1], tag="box")
        boy = load(consts, boy_d, [36, 1], tag="boy")
        bmk = load(consts, bmk_d, [36, 1], tag="bmk")
        wout = [load(consts, wout_d[0:128, :], [128, C], fp16, tag="wout0"),
                load(consts, wout_d[128:256, :], [128, C], fp16, tag="wout1")]
        bout = [load(consts, bout_d[0:128], [128, 1], tag="bout0"),
                load(consts, bout_d[128:256], [128, 1], tag="bout1")]
        smt = [load(consts, S_d[i], [36, 100], bf16, tag=f"smt{i}") for i in range(9)]
        e9 = load(consts, e9_d, [36, 4], bf16, tag="e9")
        e9t = load(consts, e9t_d, [4, 36], bf16, tag="e9t")
        e8 = load(consts, e8_d, [8, 8, 128], bf16, tag="e8")
        identb = consts.tile([128, 128], bf16, tag="identb", name="identb")
        make_identity(nc, identb)
        identh = consts.tile([128, 128], fp16, tag="identh", name="identh")
        make_identity(nc, identh)
        ones_k = consts.tile([128, 1], bf16, tag="ones_k", name="ones_k")
        nc.vector.memset(ones_k, 1.0)
        eps8 = consts.tile([8, 1], f32, tag="eps8", name="eps8")
        nc.vector.memset(eps8, 1e-5)
        b_p1 = consts.tile([36, 1], f32, tag="b_p1", name="b_p1")
        nc.vector.memset(b_p1, 1.0)
        b_m1 = consts.tile([36, 1], f32, tag="b_m1", name="b_m1")
        nc.vector.memset(b_m1, -1.0)
        if have_inb:
            inb_b = consts.tile([128, C], f32, tag="inb", name="inb")
            nc.sync.dma_start(out=inb_b, in_=bass.AP(tensor=inb_d.tensor, offset=0,
                                                     ap=[[0, 128], [1, C]]))

        with tc.tile_pool(name="pers", bufs=1) as pers:
            # persistent mid-pipeline tensors
            # val_T: partition (h, ox), h = oy//32; free (oy%32, c)  (fp16)
            val_T = pers.tile([128, 32, C], fp16, tag="valT", name="valT")
            DWT = [pers.tile([128, H, 25], f32, tag=f"DWT{pr}", name=f"DWT{pr}")
                   for pr in range(2)]
            acc = [pers.tile([128, H, GC], fp16, tag=f"acc{pr}", name=f"acc{pr}")
                   for pr in range(2)]

            with tc.tile_pool(name="psF", bufs=2, space="PSUM") as psF:
                with tc.tile_pool(name="M3", bufs=1) as M3:
                    DW = M3.tile([100, S], fp16, tag="DW", name="DW")
                    t_ = [M3.tile([128, H, W], bf16, tag=f"t{m}", name=f"t{m}")
                          for m in range(2)]

                    with tc.tile_pool(name="M1", bufs=1) as M1:
                        y = [M1.tile([128, H, W], bf16, tag=f"y{m}", name=f"y{m}")
                             for m in range(2)]
                        ypad = [M1.tile([128, 66, 66], bf16, tag=f"yp{m}", name=f"yp{m}")
                                for m in range(2)]

                        # ---- 1x1 conv (x streamed in 512-col slices, bf16) ----
                        with tc.tile_pool(name="xsP", bufs=3) as xsP:
                            for n in range(NB):
                                sl = slice(n * 512, (n + 1) * 512)
                                xs0 = load(xsP, x_d[0:128, sl], [128, 512], bf16, tag="xs0")
                                xs1 = load(xsP, x_d[128:192, sl], [64, 512], bf16, tag="xs1")
                                for mt in range(2):
                                    ps = psF.tile([128, 512], f32, tag="ps", name="ps")
                                    nc.tensor.matmul(ps, wc[0][:, mt * 128:(mt + 1) * 128], xs0, start=True, stop=False)
                                    nc.tensor.matmul(ps, wc[1][:, mt * 128:(mt + 1) * 128], xs1, start=False, stop=True)
                                    nc.scalar.activation(flat(y[mt])[:, sl], ps, AF.Identity, bias=bc[mt])

                        # ---- ypad + depthwise conv -> t (bf16) ----
                        for mt in range(2):
                            nc.gpsimd.memset(ypad[mt], 0.0)
                            nc.vector.tensor_copy(ypad[mt][:, 1:65, 1:65], y[mt])
                        for mt in range(2):
                            for n in range(NB):
                                ps = psF.tile([128, 8, 64], f32, tag="ps", name="ps")
                                oy0 = n * 8
                                for tap in range(9):
                                    ky, kx = tap // 3, tap % 3
                                    nc.tensor.matmul(ps, dwd[:, tap, mt, :],
                                                     ypad[mt][:, oy0 + ky:oy0 + ky + 8, kx:kx + 64],
                                                     start=(tap == 0), stop=(tap == 8))
                                nc.scalar.activation(t_[mt][:, oy0:oy0 + 8, :], ps, AF.Identity, bias=bdw[mt])

                        # ---- in_proj -> val_T (fp16, two oy-halves via psum halves) ----
                        for oy in range(H):
                            h = oy // 32
                            ps = psF.tile([128, C], f32, tag="ps", name="ps")
                            po = ps[h * 64:(h + 1) * 64, :]
                            nc.tensor.matmul(po, y[0][:, oy, :], win[0], start=True, stop=False)
                            nc.tensor.matmul(po, y[1][:, oy, :], win[1], start=False, stop=True)
                            nc.scalar.activation(val_T[h * 64:(h + 1) * 64, oy % 32, :], po, AF.Identity)
                        if have_inb:
                            bcast = bass.AP(tensor=inb_b.tensor, offset=inb_b.offset,
                                            ap=[inb_b.ap[0], [0, 32], [1, C]])
                            nc.vector.tensor_add(val_T, val_T, bcast)


                    # ---- M2: LN stats + normalize + offsets/masks + DW/FMA pipeline ----
                    with tc.tile_pool(name="M2", bufs=1) as M2:
                        sA = M2.tile([8, 512], f32, tag="sA", name="sA")   # mean -> mean*rstd
                        sB = M2.tile([8, 512], f32, tag="sB", name="sB")   # E[t^2] -> var
                        sD = M2.tile([8, 512], f32, tag="sD", name="sD")   # mean^2 -> rstd
                        sC = sD
                        sDb = M2.tile([8, 512], bf16, tag="sDb", name="sDb")
                        sAb = M2.tile([8, 512], bf16, tag="sAb", name="sAb")
                        with tc.tile_pool(name="sqP", bufs=3) as sqP:
                            for (isq, dst8) in ((0, sA), (1, sB)):
                                for n in range(NB):
                                    sl = slice(n * 512, (n + 1) * 512)
                                    ps = psF.tile([1, 512], f32, tag="ps", name="ps")
                                    if isq:
                                        for mt in range(2):
                                            tq = sqP.tile([128, 512], bf16, tag="tq", name="tq")
                                            nc.scalar.activation(tq, flat(t_[mt])[:, sl], AF.Square)
                                            nc.tensor.matmul(ps, ones_k, tq, start=(mt == 0), stop=(mt == 1))
                                    else:
                                        nc.tensor.matmul(ps, ones_k, flat(t_[0])[:, sl], start=True, stop=False)
                                        nc.tensor.matmul(ps, ones_k, flat(t_[1])[:, sl], start=False, stop=True)
                                    stg = sqP.tile([1, 512], f32, tag="stg", name="stg")
                                    nc.vector.tensor_copy(stg, ps)
                                    nc.sync.dma_start(out=dst8[n:n + 1, :], in_=stg)
                        nc.scalar.mul(sA, sA, 1.0 / C)
                        nc.scalar.mul(sB, sB, 1.0 / C)
                        nc.scalar.activation(sC, sA, AF.Square)
                        nc.vector.scalar_tensor_tensor(sB, sC, -1.0, sB, op0=ALU.mult, op1=ALU.add)
                        nc.scalar.activation(sB, sB, AF.Identity, bias=eps8)
                        nc.vector.reciprocal(sB, sB)
                        nc.scalar.activation(sD, sB, AF.Sqrt)
                        nc.vector.tensor_mul(sA, sA, sD)
                        nc.vector.tensor_copy(sDb, sD)
                        nc.vector.tensor_copy(sAb, sA)

                        # normalize + gelu -> in-place into t_ (bf16)
                        ta = t_
                        with tc.tile_pool(name="uP", bufs=3) as uP:
                            for n in range(NB):
                                sl = slice(n * 512, (n + 1) * 512)
                                ps1 = psF.tile([128, 512], f32, tag="ps", name="ps")
                                ps2 = psF.tile([128, 512], f32, tag="ps", name="ps")
                                nc.tensor.matmul(ps1, e8[:, n, :], sDb, start=True, stop=True)
                                nc.tensor.matmul(ps2, e8[:, n, :], sAb, start=True, stop=True)
                                for mt in range(2):
                                    u = uP.tile([128, 512], f32, tag="u", name="u")
                                    nc.vector.tensor_mul(u, flat(t_[mt])[:, sl], ps1)
                                    nc.vector.tensor_sub(u, u, ps2)
                                    nc.scalar.activation(flat(ta[mt])[:, sl], u, AF.Gelu, bias=lnb[mt], scale=lng[mt])

                        # ---- chunked pipeline: offsets/masks/DW -> DWT -> FMA
                        #      -> transpose-out -> out_proj, per 16-row chunk ----
                        SC = 1024
                        with tc.tile_pool(name="vxP", bufs=1) as vxP, \
                             tc.tile_pool(name="tbP", bufs=2) as tbP, \
                             tc.tile_pool(name="E1", bufs=2) as E1, \
                             tc.tile_pool(name="psT", bufs=2, space="PSUM") as psT:
                            # vxc buffers persist across chunks (bufs=2 alternate);
                            # only interior rows get rewritten each chunk, edge
                            # zeros from the initial memset persist.
                            vxc_bufs = []
                            for bi in range(2):
                                vb = [[vxP.tile([128, CHH + 4, GC], fp16,
                                                tag=f"vx{bi}_{pr}_{dxi}",
                                                name=f"vx{bi}_{pr}_{dxi}")
                                       for dxi in range(5)] for pr in range(2)]
                                for pr in range(2):
                                    for dxi in range(5):
                                        nc.gpsimd.memset(vb[pr][dxi], 0.0)
                                vxc_bufs.append(vb)

                            for ci in range(NCH):
                                oy0 = ci * CHH
                                sl_c = slice(ci * SC, (ci + 1) * SC)
                                # --- offsets / masks / hats / DW for this chunk ---
                                oxt = M2.tile([36, SC], bf16, tag="oxt", name="oxt")
                                oyt = M2.tile([36, SC], bf16, tag="oyt", name="oyt")
                                ex = M2.tile([36, SC], bf16, tag="ex", name="ex")
                                for nb2 in range(2):
                                    n = ci * 2 + nb2
                                    sl = slice(n * 512, (n + 1) * 512)
                                    cl = slice(nb2 * 512, (nb2 + 1) * 512)
                                    for (wgt, bia, dst2, fn) in ((wox, box, oxt, AF.Identity),
                                                                 (woy, boy, oyt, AF.Identity),
                                                                 (wmk, bmk, ex, AF.Exp)):
                                        ps = psF.tile([36, 512], f32, tag="ps", name="ps")
                                        nc.tensor.matmul(ps, wgt[0], flat(ta[0])[:, sl], start=True, stop=False)
                                        nc.tensor.matmul(ps, wgt[1], flat(ta[1])[:, sl], start=False, stop=True)
                                        nc.scalar.activation(dst2[:, cl], ps, fn, bias=bia)
                                rm = M2.tile([4, SC], bf16, tag="rm", name="rm")
                                mask = M2.tile([36, SC], bf16, tag="mask", name="mask")
                                for nb2 in range(2):
                                    cl = slice(nb2 * 512, (nb2 + 1) * 512)
                                    ps = psF.tile([4, 512], f32, tag="ps", name="ps")
                                    nc.tensor.matmul(ps, e9, ex[:, cl], start=True, stop=True)
                                    rmf = M2.tile([4, 512], f32, tag="rmf", name="rmf")
                                    nc.vector.reciprocal(rmf, ps)
                                    nc.vector.tensor_copy(rm[:, cl], rmf)
                                for nb2 in range(2):
                                    cl = slice(nb2 * 512, (nb2 + 1) * 512)
                                    ps = psF.tile([36, 512], f32, tag="ps", name="ps")
                                    nc.tensor.matmul(ps, e9t, rm[:, cl], start=True, stop=True)
                                    nc.vector.tensor_mul(mask[:, cl], ex[:, cl], ps)

                                def hats(src2, pfx):
                                    out3 = []
                                    for (kk, off) in (("m", b_p1), ("c", None), ("p", b_m1)):
                                        ab = M2.tile([36, SC], bf16, tag="hab", name="hab")
                                        if off is None:
                                            nc.scalar.activation(ab, src2, AF.Abs)
                                        else:
                                            nc.scalar.activation(ab, src2, AF.Abs, bias=off)
                                        h = M2.tile([36, SC], bf16, tag=f"h{pfx}{kk}", name=f"h{pfx}{kk}")
                                        nc.scalar.activation(h, ab, AF.Relu, bias=b_p1, scale=-1.0)
                                        out3.append(h)
                                    return out3
                                hx3 = hats(oxt, "x")
                                hy3 = hats(oyt, "y")
                                for yb in range(3):
                                    nc.gpsimd.tensor_mul(hy3[yb], mask, hy3[yb])  # hy -> m*hy
                                psds = [psF.tile([100, 512], f32, tag=f"dwp{i}",
                                                 name=f"dwp{i}", bufs=1) for i in range(2)]
                                for xb in range(3):
                                    for yb in range(3):
                                        ki = xb * 3 + yb
                                        txb = M2.tile([36, SC], bf16, tag="txb", name="txb")
                                        nc.gpsimd.tensor_mul(txb, hy3[yb], hx3[xb])
                                        for nb2 in range(2):
                                            cl = slice(nb2 * 512, (nb2 + 1) * 512)
                                            nc.tensor.matmul(psds[nb2], smt[ki], txb[:, cl],
                                                             start=(ki == 0), stop=(ki == 8))
                                for nb2 in range(2):
                                    n = ci * 2 + nb2
                                    nc.scalar.activation(DW[:, n * 512:(n + 1) * 512], psds[nb2], AF.Identity)

                                # --- DW chunk -> DWT via PE transposes + remap DMA ---
                                tbuf = tbP.tile([128, 8, 100], f32, tag="tbuf", name="tbuf")
                                for tch in range(8):
                                    gch = ci * 8 + tch
                                    ps = psT.tile([128, 128], fp16, tag="tr", name="tr")
                                    nc.tensor.transpose(ps[:, 0:100], DW[:, gch * 128:(gch + 1) * 128], identh[0:100, 0:100])
                                    nc.vector.tensor_copy(tbuf[:, tch, :], ps[:, 0:100])
                                for pr in range(2):
                                    for g2 in range(2):
                                        g = pr * 2 + g2
                                        for par in range(2):
                                            d0 = DWT[pr][g2 * 64:(g2 + 1) * 64, :, :]
                                            dst = bass.AP(tensor=d0.tensor,
                                                          offset=d0.offset + (oy0 + par) * 25,
                                                          ap=[d0.ap[0], [50, 8], [1, 25]])
                                            s0 = tbuf[par * 64:(par + 1) * 64, :, :]
                                            src = bass.AP(tensor=s0.tensor, offset=s0.offset + g * 25,
                                                          ap=[s0.ap[0], [100, 8], [1, 25]])
                                            nc.sync.dma_start(out=dst, in_=src)

                                # --- FMA chunk: load shifted value slices, 25-bin STT ---
                                vxc = vxc_bufs[ci % 2]
                                vy_lo = max(0, oy0 - 2)
                                vy_hi = min(H, oy0 + CHH + 2)
                                for pr in range(2):
                                    for dxi in range(5):
                                        dx = dxi - 2
                                        for g2 in range(2):
                                            g = pr * 2 + g2
                                            lo = max(0, -dx)
                                            hi = min(64, 64 - dx)
                                            for (a, b) in ((vy_lo, min(vy_hi, 32)), (max(vy_lo, 32), vy_hi)):
                                                if a >= b:
                                                    continue
                                                h = a // 32
                                                dst = vxc[pr][dxi][g2 * 64 + lo:g2 * 64 + hi,
                                                                   a + 2 - oy0:b + 2 - oy0, :]
                                                src = val_T[h * 64 + lo + dx:h * 64 + hi + dx,
                                                            a - h * 32:b - h * 32,
                                                            g * GC:(g + 1) * GC]
                                                nc.sync.dma_start(out=dst, in_=src)
                                        # zero rows outside the copied band (stale data
                                        # from the other chunk sharing this buffer)
                                        if vy_lo > oy0 - 2:
                                            nc.gpsimd.memset(vxc[pr][dxi][:, 0:vy_lo - (oy0 - 2), :], 0.0)
                                        if vy_hi < oy0 + CHH + 2:
                                            nc.gpsimd.memset(
                                                vxc[pr][dxi][:, vy_hi - (oy0 - 2):CHH + 4, :], 0.0)
                                for pr in range(2):
                                    for oyl in range(CHH):
                                        oy = oy0 + oyl
                                        eng = nc.vector
                                        first = True
                                        for dyi in range(5):
                                            for dxi in range(5):
                                                d = dyi * 5 + dxi
                                                sc = DWT[pr][:, oy, d:d + 1]
                                                v = vxc[pr][dxi][:, oyl + dyi, :]
                                                o = acc[pr][:, oy, :]
                                                if first:
                                                    eng.tensor_scalar_mul(o, v, sc)
                                                    first = False
                                                else:
                                                    eng.scalar_tensor_tensor(o, v, sc, o, op0=ALU.mult, op1=ALU.add)

                                # --- transpose acc chunk back + out_proj + store ---
                                RO = [E1.tile([128, CHH, W], fp16, tag=f"ro{pr}", name=f"ro{pr}")
                                      for pr in range(2)]
                                tb2 = E1.tile([128, 8, 128], fp16, tag="tb2", name="tb2")
                                for pr in range(2):
                                    for tch in range(8):
                                        ps = psT.tile([128, 128], fp16, tag="tr", name="tr")
                                        nc.tensor.transpose(ps, flat(acc[pr])[:, ci * SC + tch * 128:ci * SC + (tch + 1) * 128], identh)
                                        nc.scalar.activation(tb2[:, tch, :], ps, AF.Identity)
                                    for g2 in range(2):
                                        for par in range(2):
                                            d0 = RO[pr][g2 * 64:(g2 + 1) * 64, :, :]
                                            dst = bass.AP(tensor=d0.tensor, offset=d0.offset + par * 64,
                                                          ap=[d0.ap[0], [128, 8], [1, 64]])
                                            s0 = tb2[par * 64:(par + 1) * 64, :, :]
                                            src = bass.AP(tensor=s0.tensor, offset=s0.offset + g2 * 64,
                                                          ap=[s0.ap[0], [128, 8], [1, 64]])
                                            nc.sync.dma_start(out=dst, in_=src)

                                for mt in range(2):
                                    for n2 in range(2):
                                        sl = slice(ci * SC + n2 * 512, ci * SC + (n2 + 1) * 512)
                                        cl = slice(n2 * 512, (n2 + 1) * 512)
                                        ps = psF.tile([128, 512], f32, tag="ops", name="ops")
                                        nc.tensor.matmul(ps, wout[0][:, mt * 128:(mt + 1) * 128],
                                                         flat(RO[0])[:, cl], start=True, stop=False)
                                        nc.tensor.matmul(ps, wout[1][:, mt * 128:(mt + 1) * 128],
                                                         flat(RO[1])[:, cl], start=False, stop=True)
                                        osb = E1.tile([128, 512], f32, tag="osb", name="osb", bufs=2)
                                        nc.scalar.activation(osb, ps, AF.Identity, bias=bout[mt])
                                        nc.sync.dma_start(out=out_d[mt * 128:(mt + 1) * 128, sl], in_=osb)


def _get_program(have_inb):
    key = ("prog", have_inb)
    if key not in _CACHE:
        import concourse.bacc as bacc
        import concourse.tile as tile
        nc = bacc.Bacc("TRN2", target_bir_lowering=False, debug=False,
                       enable_asserts=False)
        with tile.TileContext(nc) as tc:
            _build(nc, tc, have_inb)
        nc.compile()
        _CACHE[key] = nc
    return _CACHE[key]


def kernel(**inputs):
    import ml_dtypes
    inputs = {k: np.asarray(v) for k, v in inputs.items()}
    w = _prep_weights(inputs)
    have_inb = bool(np.any(w['inb']))
    nc = _get_program(have_inb)

    base = {
        'wc': w['wc'], 'bc': w['bc'], 'win': w['win'], 'dwd': w['dwd'],
        'bdw': w['bdw'], 'lng': w['ln_g'], 'lnb': w['ln_b'],
        'wox': w['wox'], 'woy': w['woy'], 'wmk': w['wmk'],
        'box': w['box'], 'boy': w['boy'], 'bmk': w['bmk'],
        'wout': w['wout'], 'bout': w['bout'],
        'smats': w['smats'], 'e9': w['e9'], 'e9t': w['e9t'], 'e8sel': w['e8sel'],
    }
    if have_inb:
        base['inb'] = w['inb'].reshape(1, C)
    x = np.asarray(inputs['x'], np.float32).reshape(N, C_IN, S).astype(ml_dtypes.bfloat16)
    in_maps = []
    for core in range(NCORES):
        m = dict(base)
        m['x'] = np.ascontiguousarray(x[core])
        in_maps.append(m)

    from concourse import bass_utils
    res = bass_utils.run_bass_kernel_spmd(nc, in_maps, core_ids=list(range(NCORES)),
                                          trace=TRACE)
    global _LAST_EXEC_NS
    _LAST_EXEC_NS = res.exec_time_ns
    if TRACE:
        import sys
        print(f"[kernel] exec_time_ns={res.exec_time_ns} trace={res.instructions_and_trace[1] if res.instructions_and_trace else None}", file=sys.stderr)
    out = np.stack([r['out'].reshape(C, H, W) for r in res.results])
    return out.astype(np.float32)


# revision 9
# speedup vs baseline: 1.6398x; 1.0214x over previous
"""DCNv3_C Trainium2 Bass kernel.

8-core data parallelism over the batch (one image per NeuronCore).
Per core: 1x1 conv -> value proj -> depthwise 3x3 (block-diag matmuls)
-> LN+gelu -> offset/mask proj -> softmax -> dense 5x5 "hat" sampling
weights -> 25-bin weighted window sum (DVE scalar_tensor_tensor)
-> output proj.

DCNv3 bilinear sampling is rewritten exactly (for |offset|<=1) as a 5x5
locally-connected window:
  acc[s,g,c] = sum_{dy,dx in [-2,2]} DW[s,g,dy,dx] * VP[s+(dy,dx), g, c]
  DW[s,g,dy,dx] = sum_p mask_p * hat(gy_p+offy_p-dy) * hat(gx_p+offx_p-dx)
with hat(t)=max(0,1-|t|) and VP the value map zero-padded by 2.

v2: all matmul paths bf16/fp16 (x cast host-side), fp16 sampling
accumulator, LN rstd via ACT Rsqrt, softmax reciprocal on ACT, and the
whole back half (DW build -> transpose -> 25-bin FMA -> output-side
transpose -> out_proj -> store) pipelined in 4 row chunks so PE/ACT/DMA
work overlaps the DVE-bound FMA.
"""

import numpy as np

N, C_IN, C, H, W = 8, 192, 256, 64, 64
G, K, PAD = 4, 3, 1
GC = C // G          # 64
P = K * K            # 9
S = H * W            # 4096
NCORES = 8

_CACHE = {}
TRACE = False
_LAST_EXEC_NS = None


def _host_consts():
    # p = a*3+b with grid_x = a-1 (slowest), grid_y = b-1
    gx = np.repeat(np.arange(3) - 1, 3)
    gy = np.tile(np.arange(3) - 1, 3)
    # p-sum selection matrices, one per (xb, yb): [36, 100]
    # row (g, p) -> col g*25 + d, d = (dy+2)*5 + (dx+2)
    Smats = np.zeros((3, 3, 36, 100), np.float32)
    for xb in range(3):
        for yb in range(3):
            for g in range(G):
                for p_ in range(P):
                    dy = gy[p_] + yb - 1
                    dx = gx[p_] + xb - 1
                    d = (dy + 2) * 5 + (dx + 2)
                    Smats[xb, yb, g * 9 + p_, g * 25 + d] = 1.0
    E9 = np.zeros((36, 4), np.float32)     # per-group sums
    E9T = np.zeros((4, 36), np.float32)    # per-group broadcast
    for g in range(G):
        E9[g * 9:(g + 1) * 9, g] = 1.0
        E9T[g, g * 9:(g + 1) * 9] = 1.0
    return Smats, E9, E9T


def _prep_weights(inp):
    import ml_dtypes
    bf = ml_dtypes.bfloat16
    w = {}
    w['wc'] = np.ascontiguousarray(inp['conv_w'].T).astype(bf)            # [192,256]
    w['bc'] = inp['conv_b'].reshape(C, 1).astype(np.float32)
    w['win'] = np.ascontiguousarray(inp['in_w'].T).astype(bf)             # [c,o]
    w['inb'] = np.asarray(inp['in_b'], np.float32)
    # depthwise diag weights, partition-major: [128, 9, 2, 128]
    dwd = np.zeros((128, 9, 2, 128), np.float32)
    dw = inp['dw_w'].reshape(C, 9)
    for tap in range(9):
        for mt in range(2):
            for i in range(128):
                dwd[i, tap, mt, i] = dw[mt * 128 + i, tap]
    w['dwd'] = dwd.astype(bf)
    w['bdw'] = inp['dw_b'].reshape(C, 1).astype(np.float32)
    w['ln_g'] = inp['ln_g'].reshape(C, 1).astype(np.float32)
    w['ln_b'] = inp['ln_b'].reshape(C, 1).astype(np.float32)
    # offset/mask projections: wox/woy/wmk [256, 36] lhsT, col = g*9+p
    wox = np.zeros((C, 36), np.float32)
    woy = np.zeros((C, 36), np.float32)
    box = np.zeros((36, 1), np.float32)
    boy = np.zeros((36, 1), np.float32)
    ow, ob = np.asarray(inp['off_w'], np.float32), np.asarray(inp['off_b'], np.float32)
    for g in range(G):
        for p_ in range(P):
            wox[:, g * 9 + p_] = ow[g * 18 + p_ * 2 + 0]
            woy[:, g * 9 + p_] = ow[g * 18 + p_ * 2 + 1]
            box[g * 9 + p_, 0] = ob[g * 18 + p_ * 2 + 0]
            boy[g * 9 + p_, 0] = ob[g * 18 + p_ * 2 + 1]
    w['wox'], w['woy'] = wox.astype(bf), woy.astype(bf)
    w['box'], w['boy'] = box, boy
    w['wmk'] = np.ascontiguousarray(inp['mask_w'].T).astype(bf)           # [256,36]
    w['bmk'] = inp['mask_b'].reshape(36, 1).astype(np.float32)
    w['wout'] = np.ascontiguousarray(inp['out_w'].T).astype(np.float16)   # [gc,o]
    w['bout'] = inp['out_b'].reshape(C, 1).astype(np.float32)
    Smats, E9, E9T = _host_consts()
    w['smats'] = np.ascontiguousarray(Smats.reshape(9, 36, 100)).astype(bf)
    w['e9'] = E9.astype(bf)
    w['e9t'] = E9T.astype(bf)
    e8 = np.zeros((8, 8, 128), np.float32)
    for n in range(8):
        e8[n, n, :] = 1.0
    w['e8sel'] = e8.reshape(8, 1024).astype(bf)
    return w


def _build(nc, tc, have_inb):
    import concourse.bass as bass
    import concourse.mybir as mybir
    from concourse.masks import make_identity
    f32 = mybir.dt.float32
    bf16 = mybir.dt.bfloat16
    fp16 = mybir.dt.float16
    AF = mybir.ActivationFunctionType
    ALU = mybir.AluOpType

    def dram(name, shape, dt=f32, kind="ExternalInput"):
        return nc.dram_tensor(name, shape, dt, kind=kind).ap()

    x_d = dram("x", [C_IN, S], bf16)
    wc_d = dram("wc", [C_IN, C], bf16)
    bc_d = dram("bc", [C, 1])
    win_d = dram("win", [C, C], bf16)
    dwd_d = dram("dwd", [128, 9, 2, 128], bf16)
    bdw_d = dram("bdw", [C, 1])
    lng_d = dram("lng", [C, 1])
    lnb_d = dram("lnb", [C, 1])
    wox_d = dram("wox", [C, 36], bf16)
    woy_d = dram("woy", [C, 36], bf16)
    wmk_d = dram("wmk", [C, 36], bf16)
    box_d = dram("box", [36, 1])
    boy_d = dram("boy", [36, 1])
    bmk_d = dram("bmk", [36, 1])
    wout_d = dram("wout", [C, C], fp16)
    bout_d = dram("bout", [C, 1])
    S_d = dram("smats", [9, 36, 100], bf16)
    e9_d = dram("e9", [36, 4], bf16)
    e9t_d = dram("e9t", [4, 36], bf16)
    e8_d = dram("e8sel", [8, 1024], bf16)
    inb_d = dram("inb", [1, C]) if have_inb else None
    out_d = dram("out", [C, S], kind="ExternalOutput")

    def load(pool, dr, shape, dt=f32, tag=None):
        t = pool.tile(shape, dt, tag=tag, name=tag)
        nc.sync.dma_start(out=t, in_=dr)
        return t

    def flat(t):
        return t.rearrange("p a b -> p (a b)")

    NB = 8          # n-blocks of 512
    NCH = 4         # row chunks for the pipelined back half
    CHH = H // NCH  # 16 rows per chunk

    with tc.tile_pool(name="consts", bufs=1) as consts:
        wc = [load(consts, wc_d[0:128, :], [128, C], bf16, tag="wc0"),
              load(consts, wc_d[128:192, :], [64, C], bf16, tag="wc1")]
        bc = [load(consts, bc_d[0:128], [128, 1], tag="bc0"),
              load(consts, bc_d[128:256], [128, 1], tag="bc1")]
        win = [load(consts, win_d[0:128, :], [128, C], bf16, tag="win0"),
               load(consts, win_d[128:256, :], [128, C], bf16, tag="win1")]
        dwd = load(consts, dwd_d, [128, 9, 2, 128], bf16, tag="dwd")
        bdw = [load(consts, bdw_d[0:128], [128, 1], tag="bdw0"),
               load(consts, bdw_d[128:256], [128, 1], tag="bdw1")]
        lng = [load(consts, lng_d[0:128], [128, 1], tag="lng0"),
               load(consts, lng_d[128:256], [128, 1], tag="lng1")]
        lnb = [load(consts, lnb_d[0:128], [128, 1], tag="lnb0"),
               load(consts, lnb_d[128:256], [128, 1], tag="lnb1")]
        wox = [load(consts, wox_d[0:128, :], [128, 36], bf16, tag="wox0"),
               load(consts, wox_d[128:256, :], [128, 36], bf16, tag="wox1")]
        woy = [load(consts, woy_d[0:128, :], [128, 36], bf16, tag="woy0"),
               load(consts, woy_d[128:256, :], [128, 36], bf16, tag="woy1")]
        wmk = [load(consts, wmk_d[0:128, :], [128, 36], bf16, tag="wmk0"),
               load(consts, wmk_d[128:256, :], [128, 36], bf16, tag="wmk1")]
        box = load(consts, box_d, [36, 1], tag="box")
        boy = load(consts, boy_d, [36, 1], tag="boy")
        bmk = load(consts, bmk_d, [36, 1], tag="bmk")
        wout = [load(consts, wout_d[0:128, :], [128, C], fp16, tag="wout0"),
                load(consts, wout_d[128:256, :], [128, C], fp16, tag="wout1")]
        bout = [load(consts, bout_d[0:128], [128, 1], tag="bout0"),
                load(consts, bout_d[128:256], [128, 1], tag="bout1")]
        smt = [load(consts, S_d[i], [36, 100], bf16, tag=f"smt{i}") for i in range(9)]
        e9 = load(consts, e9_d, [36, 4], bf16, tag="e9")
        e9t = load(consts, e9t_d, [4, 36], bf16, tag="e9t")
        e8 = load(consts, e8_d, [8, 8, 128], bf16, tag="e8")
        identb = consts.tile([128, 128], bf16, tag="identb", name="identb")
        make_identity(nc, identb)
        identh = consts.tile([128, 128], fp16, tag="identh", name="identh")
        make_identity(nc, identh)
        ones_k = consts.tile([128, 1], bf16, tag="ones_k", name="ones_k")
        nc.vector.memset(ones_k, 1.0)
        eps8 = consts.tile([8, 1], f32, tag="eps8", name="eps8")
        nc.vector.memset(eps8, 1e-5)
        b_p1 = consts.tile([36, 1], f32, tag="b_p1", name="b_p1")
        nc.vector.memset(b_p1, 1.0)
        b_m1 = consts.tile([36, 1], f32, tag="b_m1", name="b_m1")
        nc.vector.memset(b_m1, -1.0)
        if have_inb:
            inb_b = consts.tile([128, C], f32, tag="inb", name="inb")
            nc.sync.dma_start(out=inb_b, in_=bass.AP(tensor=inb_d.tensor, offset=0,
                                                     ap=[[0, 128], [1, C]]))

        with tc.tile_pool(name="pers", bufs=1) as pers:
            # persistent mid-pipeline tensors
            # val_T: partition (h, ox), h = oy//32; free (oy%32, c)  (fp16)
            val_T = pers.tile([128, 32, C], fp16, tag="valT", name="valT")
            DWT = [pers.tile([128, H, 25], f32, tag=f"DWT{pr}", name=f"DWT{pr}")
                   for pr in range(2)]
            acc = [pers.tile([128, H, GC], fp16, tag=f"acc{pr}", name=f"acc{pr}")
                   for pr in range(2)]

            with tc.tile_pool(name="psF", bufs=2, space="PSUM") as psF:
                with tc.tile_pool(name="M3", bufs=1) as M3:
                    DW = M3.tile([100, S], fp16, tag="DW", name="DW")
                    t_ = [M3.tile([128, H, W], bf16, tag=f"t{m}", name=f"t{m}")
                          for m in range(2)]

                    with tc.tile_pool(name="M1", bufs=1) as M1:
                        y = [M1.tile([128, H, W], bf16, tag=f"y{m}", name=f"y{m}")
                             for m in range(2)]
                        ypad = [M1.tile([128, 66, 66], bf16, tag=f"yp{m}", name=f"yp{m}")
                                for m in range(2)]

                        # ---- 1x1 conv (x streamed in 512-col slices, bf16) ----
                        with tc.tile_pool(name="xsP", bufs=3) as xsP:
                            for n in range(NB):
                                sl = slice(n * 512, (n + 1) * 512)
                                xs0 = load(xsP, x_d[0:128, sl], [128, 512], bf16, tag="xs0")
                                xs1 = load(xsP, x_d[128:192, sl], [64, 512], bf16, tag="xs1")
                                for mt in range(2):
                                    ps = psF.tile([128, 512], f32, tag="ps", name="ps")
                                    nc.tensor.matmul(ps, wc[0][:, mt * 128:(mt + 1) * 128], xs0, start=True, stop=False)
                                    nc.tensor.matmul(ps, wc[1][:, mt * 128:(mt + 1) * 128], xs1, start=False, stop=True)
                                    nc.scalar.activation(flat(y[mt])[:, sl], ps, AF.Identity, bias=bc[mt])

                        # ---- ypad + depthwise conv -> t (bf16) ----
                        for mt in range(2):
                            nc.gpsimd.memset(ypad[mt], 0.0)
                            nc.vector.tensor_copy(ypad[mt][:, 1:65, 1:65], y[mt])
                        for mt in range(2):
                            for n in range(NB):
                                ps = psF.tile([128, 8, 64], f32, tag="ps", name="ps")
                                oy0 = n * 8
                                for tap in range(9):
                                    ky, kx = tap // 3, tap % 3
                                    nc.tensor.matmul(ps, dwd[:, tap, mt, :],
                                                     ypad[mt][:, oy0 + ky:oy0 + ky + 8, kx:kx + 64],
                                                     start=(tap == 0), stop=(tap == 8))
                                nc.scalar.activation(t_[mt][:, oy0:oy0 + 8, :], ps, AF.Identity, bias=bdw[mt])

                        # ---- in_proj -> val_T (fp16, two oy-halves via psum halves) ----
                        for oy in range(H):
                            h = oy // 32
                            ps = psF.tile([128, C], f32, tag="ps", name="ps")
                            po = ps[h * 64:(h + 1) * 64, :]
                            nc.tensor.matmul(po, y[0][:, oy, :], win[0], start=True, stop=False)
                            nc.tensor.matmul(po, y[1][:, oy, :], win[1], start=False, stop=True)
                            nc.scalar.activation(val_T[h * 64:(h + 1) * 64, oy % 32, :], po, AF.Identity)
                        if have_inb:
                            bcast = bass.AP(tensor=inb_b.tensor, offset=inb_b.offset,
                                            ap=[inb_b.ap[0], [0, 32], [1, C]])
                            nc.vector.tensor_add(val_T, val_T, bcast)


                    # ---- M2: LN stats + normalize + offsets/masks + DW/FMA pipeline ----
                    with tc.tile_pool(name="M2", bufs=1) as M2:
                        sA = M2.tile([8, 512], f32, tag="sA", name="sA")   # mean -> mean*rstd
                        sB = M2.tile([8, 512], f32, tag="sB", name="sB")   # E[t^2] -> var
                        sD = M2.tile([8, 512], f32, tag="sD", name="sD")   # mean^2 -> rstd
                        sC = sD
                        sDb = M2.tile([8, 512], bf16, tag="sDb", name="sDb")
                        sAb = M2.tile([8, 512], bf16, tag="sAb", name="sAb")
                        with tc.tile_pool(name="sqP", bufs=3) as sqP:
                            for (isq, dst8) in ((0, sA), (1, sB)):
                                for n in range(NB):
                                    sl = slice(n * 512, (n + 1) * 512)
                                    ps = psF.tile([1, 512], f32, tag="ps", name="ps")
                                    if isq:
                                        for mt in range(2):
                                            tq = sqP.tile([128, 512], bf16, tag="tq", name="tq")
                                            nc.scalar.activation(tq, flat(t_[mt])[:, sl], AF.Square)
                                            nc.tensor.matmul(ps, ones_k, tq, start=(mt == 0), stop=(mt == 1))
                                    else:
                                        nc.tensor.matmul(ps, ones_k, flat(t_[0])[:, sl], start=True, stop=False)
                                        nc.tensor.matmul(ps, ones_k, flat(t_[1])[:, sl], start=False, stop=True)
                                    stg = sqP.tile([1, 512], f32, tag="stg", name="stg")
                                    nc.vector.tensor_copy(stg, ps)
                                    nc.sync.dma_start(out=dst8[n:n + 1, :], in_=stg)
                        nc.scalar.mul(sA, sA, 1.0 / C)
                        nc.scalar.mul(sB, sB, 1.0 / C)
                        nc.scalar.activation(sC, sA, AF.Square)
                        nc.vector.scalar_tensor_tensor(sB, sC, -1.0, sB, op0=ALU.mult, op1=ALU.add)
                        nc.scalar.activation(sB, sB, AF.Identity, bias=eps8)
                        nc.vector.reciprocal(sB, sB)
                        nc.scalar.activation(sD, sB, AF.Sqrt)
                        nc.vector.tensor_mul(sA, sA, sD)
                        nc.vector.tensor_copy(sDb, sD)
                        nc.vector.tensor_copy(sAb, sA)

                        # normalize + gelu -> in-place into t_ (bf16)
                        ta = t_
                        with tc.tile_pool(name="uP", bufs=3) as uP:
                            for n in range(NB):
                                sl = slice(n * 512, (n + 1) * 512)
                                ps1 = psF.tile([128, 512], f32, tag="ps", name="ps")
                                ps2 = psF.tile([128, 512], f32, tag="ps", name="ps")
                                nc.tensor.matmul(ps1, e8[:, n, :], sDb, start=True, stop=True)
                                nc.tensor.matmul(ps2, e8[:, n, :], sAb, start=True, stop=True)
                                for mt in range(2):
                                    u = uP.tile([128, 512], f32, tag="u", name="u")
                                    nc.vector.tensor_mul(u, flat(t_[mt])[:, sl], ps1)
                                    nc.vector.tensor_sub(u, u, ps2)
                                    nc.scalar.activation(flat(ta[mt])[:, sl], u, AF.Gelu, bias=lnb[mt], scale=lng[mt])

                        # ---- chunked pipeline: offsets/masks/DW -> DWT -> FMA
                        #      -> transpose-out -> out_proj, per 16-row chunk ----
                        SC = 1024
                        with tc.tile_pool(name="vxP", bufs=1) as vxP, \
                             tc.tile_pool(name="tbP", bufs=2) as tbP, \
                             tc.tile_pool(name="E1", bufs=2) as E1, \
                             tc.tile_pool(name="psT", bufs=2, space="PSUM") as psT:
                            # vxc buffers persist across chunks (bufs=2 alternate);
                            # only interior rows get rewritten each chunk, edge
                            # zeros from the initial memset persist.
                            vxc_bufs = []
                            for bi in range(2):
                                vb = [[vxP.tile([128, CHH + 4, GC], fp16,
                                                tag=f"vx{bi}_{pr}_{dxi}",
                                                name=f"vx{bi}_{pr}_{dxi}")
                                       for dxi in range(5)] for pr in range(2)]
                                for pr in range(2):
                                    for dxi in range(5):
                                        nc.gpsimd.memset(vb[pr][dxi], 0.0)
                                vxc_bufs.append(vb)

                            for ci in range(NCH):
                                oy0 = ci * CHH
                                sl_c = slice(ci * SC, (ci + 1) * SC)
                                # --- offsets / masks / hats / DW for this chunk ---
                                oxt = M2.tile([36, SC], bf16, tag="oxt", name="oxt")
                                oyt = M2.tile([36, SC], bf16, tag="oyt", name="oyt")
                                ex = M2.tile([36, SC], bf16, tag="ex", name="ex")
                                for nb2 in range(2):
                                    n = ci * 2 + nb2
                                    sl = slice(n * 512, (n + 1) * 512)
                                    cl = slice(nb2 * 512, (nb2 + 1) * 512)
                                    for (wgt, bia, dst2, fn) in ((wox, box, oxt, AF.Identity),
                                                                 (woy, boy, oyt, AF.Identity),
                                                                 (wmk, bmk, ex, AF.Exp)):
                                        ps = psF.tile([36, 512], f32, tag="ps", name="ps")
                                        nc.tensor.matmul(ps, wgt[0], flat(ta[0])[:, sl], start=True, stop=False)
                                        nc.tensor.matmul(ps, wgt[1], flat(ta[1])[:, sl], start=False, stop=True)
                                        nc.scalar.activation(dst2[:, cl], ps, fn, bias=bia)
                                rm = M2.tile([4, SC], bf16, tag="rm", name="rm")
                                mask = M2.tile([36, SC], bf16, tag="mask", name="mask")
                                for nb2 in range(2):
                                    cl = slice(nb2 * 512, (nb2 + 1) * 512)
                                    ps = psF.tile([4, 512], f32, tag="ps", name="ps")
                                    nc.tensor.matmul(ps, e9, ex[:, cl], start=True, stop=True)
                                    rmf = M2.tile([4, 512], f32, tag="rmf", name="rmf")
                                    nc.vector.reciprocal(rmf, ps)
                                    nc.vector.tensor_copy(rm[:, cl], rmf)
                                for nb2 in range(2):
                                    cl = slice(nb2 * 512, (nb2 + 1) * 512)
                                    ps = psF.tile([36, 512], f32, tag="ps", name="ps")
                                    nc.tensor.matmul(ps, e9t, rm[:, cl], start=True, stop=True)
                                    nc.vector.tensor_mul(mask[:, cl], ex[:, cl], ps)

                                def hats(src2, pfx):
                                    out3 = []
                                    for (kk, off) in (("m", b_p1), ("c", None), ("p", b_m1)):
                                        ab = M2.tile([36, SC], bf16, tag="hab", name="hab")
                                        if off is None:
                                            nc.scalar.activation(ab, src2, AF.Abs)
                                        else:
                                            nc.scalar.activation(ab, src2, AF.Abs, bias=off)
                                        h = M2.tile([36, SC], bf16, tag=f"h{pfx}{kk}", name=f"h{pfx}{kk}")
                                        nc.scalar.activation(h, ab, AF.Relu, bias=b_p1, scale=-1.0)
                                        out3.append(h)
                                    return out3
                                hx3 = hats(oxt, "x")
                                hy3 = hats(oyt, "y")
                                for yb in range(3):
                                    nc.vector.tensor_mul(hy3[yb], mask, hy3[yb])  # hy -> m*hy
                                psds = [psF.tile([100, 512], f32, tag=f"dwp{i}",
                                                 name=f"dwp{i}", bufs=1) for i in range(2)]
                                for xb in range(3):
                                    for yb in range(3):
                                        ki = xb * 3 + yb
                                        txb = M2.tile([36, SC], bf16, tag="txb", name="txb")
                                        nc.vector.tensor_mul(txb, hy3[yb], hx3[xb])
                                        for nb2 in range(2):
                                            cl = slice(nb2 * 512, (nb2 + 1) * 512)
                                            nc.tensor.matmul(psds[nb2], smt[ki], txb[:, cl],
                                                             start=(ki == 0), stop=(ki == 8))
                                for nb2 in range(2):
                                    n = ci * 2 + nb2
                                    nc.scalar.activation(DW[:, n * 512:(n + 1) * 512], psds[nb2], AF.Identity)

                                # --- DW chunk -> DWT via PE transposes + remap DMA ---
                                tbuf = tbP.tile([128, 8, 100], f32, tag="tbuf", name="tbuf")
                                for tch in range(8):
                                    gch = ci * 8 + tch
                                    ps = psT.tile([128, 128], fp16, tag="tr", name="tr")
                                    nc.tensor.transpose(ps[:, 0:100], DW[:, gch * 128:(gch + 1) * 128], identh[0:100, 0:100])
                                    nc.vector.tensor_copy(tbuf[:, tch, :], ps[:, 0:100])
                                for pr in range(2):
                                    for g2 in range(2):
                                        g = pr * 2 + g2
                                        for par in range(2):
                                            d0 = DWT[pr][g2 * 64:(g2 + 1) * 64, :, :]
                                            dst = bass.AP(tensor=d0.tensor,
                                                          offset=d0.offset + (oy0 + par) * 25,
                                                          ap=[d0.ap[0], [50, 8], [1, 25]])
                                            s0 = tbuf[par * 64:(par + 1) * 64, :, :]
                                            src = bass.AP(tensor=s0.tensor, offset=s0.offset + g * 25,
                                                          ap=[s0.ap[0], [100, 8], [1, 25]])
                                            nc.sync.dma_start(out=dst, in_=src)

                                # --- FMA chunk: load shifted value slices, 25-bin STT ---
                                vxc = vxc_bufs[ci % 2]
                                vy_lo = max(0, oy0 - 2)
                                vy_hi = min(H, oy0 + CHH + 2)
                                for pr in range(2):
                                    for dxi in range(5):
                                        dx = dxi - 2
                                        for g2 in range(2):
                                            g = pr * 2 + g2
                                            lo = max(0, -dx)
                                            hi = min(64, 64 - dx)
                                            for (a, b) in ((vy_lo, min(vy_hi, 32)), (max(vy_lo, 32), vy_hi)):
                                                if a >= b:
                                                    continue
                                                h = a // 32
                                                dst = vxc[pr][dxi][g2 * 64 + lo:g2 * 64 + hi,
                                                                   a + 2 - oy0:b + 2 - oy0, :]
                                                src = val_T[h * 64 + lo + dx:h * 64 + hi + dx,
                                                            a - h * 32:b - h * 32,
                                                            g * GC:(g + 1) * GC]
                                                nc.sync.dma_start(out=dst, in_=src)
                                        # zero rows outside the copied band (stale data
                                        # from the other chunk sharing this buffer)
                                        if vy_lo > oy0 - 2:
                                            nc.gpsimd.memset(vxc[pr][dxi][:, 0:vy_lo - (oy0 - 2), :], 0.0)
                                        if vy_hi < oy0 + CHH + 2:
                                            nc.gpsimd.memset(
                                                vxc[pr][dxi][:, vy_hi - (oy0 - 2):CHH + 4, :], 0.0)
                                for pr in range(2):
                                    for oyl in range(CHH):
                                        oy = oy0 + oyl
                                        eng = nc.vector
                                        first = True
                                        for dyi in range(5):
                                            for dxi in range(5):
                                                d = dyi * 5 + dxi
                                                sc = DWT[pr][:, oy, d:d + 1]
                                                v = vxc[pr][dxi][:, oyl + dyi, :]
                                                o = acc[pr][:, oy, :]
                                                if first:
                                                    eng.tensor_scalar_mul(o, v, sc)
                                                    first = False
                                                else:
                                                    eng.scalar_tensor_tensor(o, v, sc, o, op0=ALU.mult, op1=ALU.add)

                                # --- transpose acc chunk back + out_proj + store ---
                                RO = [E1.tile([128, CHH, W], fp16, tag=f"ro{pr}", name=f"ro{pr}")
                                      for pr in range(2)]
                                tb2 = E1.tile([128, 8, 128], fp16, tag="tb2", name="tb2")
                                for pr in range(2):
                                    for tch in range(8):
                                        ps = psT.tile([128, 128], fp16, tag="tr", name="tr")
                                        nc.tensor.transpose(ps, flat(acc[pr])[:, ci * SC + tch * 128:ci * SC + (tch + 1) * 128], identh)
                                        nc.scalar.activation(tb2[:, tch, :], ps, AF.Identity)
                                    for g2 in range(2):
                                        for par in range(2):
                                            d0 = RO[pr][g2 * 64:(g2 + 1) * 64, :, :]
                                            dst = bass.AP(tensor=d0.tensor, offset=d0.offset + par * 64,
                                                          ap=[d0.ap[0], [128, 8], [1, 64]])
                                            s0 = tb2[par * 64:(par + 1) * 64, :, :]
                                            src = bass.AP(tensor=s0.tensor, offset=s0.offset + g2 * 64,
                                                          ap=[s0.ap[0], [128, 8], [1, 64]])
                                            nc.sync.dma_start(out=dst, in_=src)

                                for mt in range(2):
                                    for n2 in range(2):
                                        sl = slice(ci * SC + n2 * 512, ci * SC + (n2 + 1) * 512)
                                        cl = slice(n2 * 512, (n2 + 1) * 512)
                                        ps = psF.tile([128, 512], f32, tag="ops", name="ops")
                                        nc.tensor.matmul(ps, wout[0][:, mt * 128:(mt + 1) * 128],
                                                         flat(RO[0])[:, cl], start=True, stop=False)
                                        nc.tensor.matmul(ps, wout[1][:, mt * 128:(mt + 1) * 128],
                                                         flat(RO[1])[:, cl], start=False, stop=True)
                                        osb = E1.tile([128, 512], f32, tag="osb", name="osb", bufs=2)
                                        nc.scalar.activation(osb, ps, AF.Identity, bias=bout[mt])
                                        nc.sync.dma_start(out=out_d[mt * 128:(mt + 1) * 128, sl], in_=osb)


def _get_program(have_inb):
    key = ("prog", have_inb)
    if key not in _CACHE:
        import concourse.bacc as bacc
        import concourse.tile as tile
        nc = bacc.Bacc("TRN2", target_bir_lowering=False, debug=False,
                       enable_asserts=False)
        with tile.TileContext(nc) as tc:
            _build(nc, tc, have_inb)
        nc.compile()
        _CACHE[key] = nc
    return _CACHE[key]


def kernel(**inputs):
    import ml_dtypes
    inputs = {k: np.asarray(v) for k, v in inputs.items()}
    w = _prep_weights(inputs)
    have_inb = bool(np.any(w['inb']))
    nc = _get_program(have_inb)

    base = {
        'wc': w['wc'], 'bc': w['bc'], 'win': w['win'], 'dwd': w['dwd'],
        'bdw': w['bdw'], 'lng': w['ln_g'], 'lnb': w['ln_b'],
        'wox': w['wox'], 'woy': w['woy'], 'wmk': w['wmk'],
        'box': w['box'], 'boy': w['boy'], 'bmk': w['bmk'],
        'wout': w['wout'], 'bout': w['bout'],
        'smats': w['smats'], 'e9': w['e9'], 'e9t': w['e9t'], 'e8sel': w['e8sel'],
    }
    if have_inb:
        base['inb'] = w['inb'].reshape(1, C)
    x = np.asarray(inputs['x'], np.float32).reshape(N, C_IN, S).astype(ml_dtypes.bfloat16)
    in_maps = []
    for core in range(NCORES):
        m = dict(base)
        m['x'] = np.ascontiguousarray(x[core])
        in_maps.append(m)

    from concourse import bass_utils
    res = bass_utils.run_bass_kernel_spmd(nc, in_maps, core_ids=list(range(NCORES)),
                                          trace=TRACE)
    global _LAST_EXEC_NS
    _LAST_EXEC_NS = res.exec_time_ns
    if TRACE:
        import sys
        print(f"[kernel] exec_time_ns={res.exec_time_ns} trace={res.instructions_and_trace[1] if res.instructions_and_trace else None}", file=sys.stderr)
    out = np.stack([r['out'].reshape(C, H, W) for r in res.results])
    return out.astype(np.float32)
